# revision 2
# baseline (speedup 1.0000x reference)
"""8-core Trainium2 Bass kernel for causal MHA — fp8 DoubleRow edition.

Sharding: core c = (batch c//2, head-half c%2); each core computes 8 heads over
the full 2048-token sequence and a partial output projection (contraction over
its 512 local columns); the host sums the two fp16 partials per batch + bias.

Numerics:
- Host scales all of Wq/Wk/Wv by 32 so fp8 residuals stay above e4m3's
  subnormal floor; projections run 3 DoubleRow passes (x8*W8, x8*dW8, dx8*W8)
  with fp32 PSUM accumulation.
- K stored as (K8, dK8) in the DoubleRow fold -> requant-exact scores;
  Q requantized at sigma~1.2 via a scaled copy. Slot-1 row 63/127 of K holds
  -8 and of Q holds o(q)*3.327: the matmul itself subtracts the per-query
  range offset o(q) = 11.54*sqrt(2 ln(q+1)).
- exp via the Schraudolph bit trick: one tensor_scalar (mult M, add 56) to
  uint8 on DVE/Pool, bitcast to fp8e4m3; causal masking inside the diagonal
  block via an extra fp8 "ramp" matmul accumulated into the same PSUM.
- AV in DoubleRow with the fold carrying (V8, dV8); ones-column 32.0 yields
  the softmax normalizer; context normalized to fp16.
- Query block 0 (tokens 0-127) runs a full fp16 path (its rows see no
  softmax averaging, so fp8 noise would land unattenuated).
"""

import sys
import numpy as np

if "/opt/trn_rl_repo" not in sys.path:
    sys.path.insert(0, "/opt/trn_rl_repo")

import ml_dtypes
import concourse.bass as bass
import concourse.tile as tile
from concourse import bacc, mybir

F32 = mybir.dt.float32
F16 = mybir.dt.float16
F8 = mybir.dt.float8e4
U8 = mybir.dt.uint8
EXP = mybir.ActivationFunctionType.Exp
COPY = mybir.ActivationFunctionType.Copy
DR = mybir.MatmulPerfMode.DoubleRow
ADD = mybir.AluOpType.add
MULT = mybir.AluOpType.mult
SUB = mybir.AluOpType.subtract

P = 128
S, D, HLOC, HD = 2048, 1024, 8, 64
NT = S // P                # 16 token tiles
NCG = 4                    # col groups (head pairs) per core
A_SCH = 8 * np.log2(np.e)  # 11.5416
CQ = 1.2 / 32.0            # Q8 = 1.2 q
MCONV = float(A_SCH / 8.0 / (1.2 * 32.0))   # 0.0375696
ESC = 1.0 / 8192.0         # early fp16: psum16 = 8192 s
ACTSC = float(MCONV / 11.5416023862437)   # ACT exp scale: psum*ACTSC = s - o
NP8 = ml_dtypes.float8_e4m3


def build_program(num_devices: int = 8, replicate: int = 1) -> bass.Bass:
    nc = bacc.Bacc("TRN2", target_bir_lowering=False, debug=False,
                   num_devices=num_devices)
    x8f = nc.dram_tensor("x8f", [P, 4, 2, S], F8, kind="ExternalInput")
    dx8f = nc.dram_tensor("dx8f", [P, 4, 2, S], F8, kind="ExternalInput")
    wq8 = nc.dram_tensor("wq8", [P, 4, 2, 512], F8, kind="ExternalInput")
    wk8 = nc.dram_tensor("wk8", [P, 4, 2, 512], F8, kind="ExternalInput")
    wv8 = nc.dram_tensor("wv8", [P, 4, 2, 512], F8, kind="ExternalInput")
    dwq8 = nc.dram_tensor("dwq8", [P, 4, 2, 512], F8, kind="ExternalInput")
    dwk8 = nc.dram_tensor("dwk8", [P, 4, 2, 512], F8, kind="ExternalInput")
    dwv8 = nc.dram_tensor("dwv8", [P, 4, 2, 512], F8, kind="ExternalInput")
    wo16d = nc.dram_tensor("wo16", [P, 4, D], F16, kind="ExternalInput")
    krow = nc.dram_tensor("krow", [1, S], F8, kind="ExternalInput")    # -8
    orow = nc.dram_tensor("orow", [1, S], F8, kind="ExternalInput")    # o*3.327
    zrows = nc.dram_tensor("zrows", [64, 2, S], F8, kind="ExternalInput")
    rampA = nc.dram_tensor("rampA", [P, 2, P], F8, kind="ExternalInput")
    rampB = nc.dram_tensor("rampB", [P, 2, P], F8, kind="ExternalInput")
    mask16 = nc.dram_tensor("mask16", [P, P], F16, kind="ExternalInput")
    out = nc.dram_tensor("out", [S, D], F16, kind="ExternalOutput")

    with tile.TileContext(nc) as tc:
        with (
            tc.tile_pool(name="res", bufs=1) as res,
            tc.tile_pool(name="work", bufs=4) as work,
            tc.tile_pool(name="exp8", bufs=6) as exp8,
            tc.tile_pool(name="obp", bufs=2) as obp,
            tc.tile_pool(name="pproj", bufs=2, space="PSUM") as pproj,
            tc.tile_pool(name="psc", bufs=3, space="PSUM") as psc,
            tc.tile_pool(name="pacc", bufs=2, space="PSUM") as pacc,
            tc.tile_pool(name="pout", bufs=1, space="PSUM") as pout,
        ):
          for _rep in range(replicate):
            # ---------- resident tensors ----------
            KT8 = res.tile([P, NCG, 2, S], F8, tag="kt8", name="KT8")
            QT8 = res.tile([P, HLOC, 2, S], F8, tag="qt8", name="QT8")
            Vall = res.tile([P, NT, 2, HLOC, 80], F8, tag="vall", name="Vall")
            ctx16 = res.tile([P, 4, S], F16, tag="ctx16", name="ctx16")
            x8s = res.tile([P, 4, 2, S], F8, tag="x8s", name="x8s")
            dx8s = res.tile([P, 4, 2, S], F8, tag="dx8s", name="dx8s")
            w8 = {}
            for nm, dt_ in [("wq8", wq8), ("wk8", wk8), ("wv8", wv8),
                            ("dwq8", dwq8), ("dwk8", dwk8), ("dwv8", dwv8)]:
                w8[nm] = res.tile([P, 4, 2, 512], F8, tag=nm, name=nm + "_s")
                nc.sync.dma_start(out=w8[nm], in_=dt_[:])
            wo16 = res.tile([P, 4, D], F16, tag="wo16", name="wo16_s")
            nc.sync.dma_start(out=wo16, in_=wo16d[:])
            nc.sync.dma_start(out=x8s, in_=x8f[:])
            nc.sync.dma_start(out=dx8s, in_=dx8f[:])
            ramps = res.tile([P, 2, 2, P], F8, tag="ramps", name="ramps")
            nc.sync.dma_start(out=ramps[:, 0, :, :], in_=rampA[:])
            nc.sync.dma_start(out=ramps[:, 1, :, :], in_=rampB[:])
            m16 = res.tile([P, P], F16, tag="m16", name="m16")
            nc.sync.dma_start(out=m16, in_=mask16[:])
            # zero bands of QT8 (head h: other-head half zeroed), via zeros DMA
            for h in range(HLOC):
                if h % 2 == 0:
                    nc.sync.dma_start(out=QT8[64:128, h, :, :], in_=zrows[:])
                else:
                    nc.sync.dma_start(out=QT8[0:64, h, :, :], in_=zrows[:])
            # K slot1 const rows (-8) and Q slot1 o-rows
            for cg in range(NCG):
                nc.sync.dma_start(out=KT8[63:64, cg, 1, :], in_=krow[:])
                nc.sync.dma_start(out=KT8[127:128, cg, 1, :], in_=krow[:])
            for h in range(HLOC):
                r = 63 if h % 2 == 0 else 127
                nc.sync.dma_start(out=QT8[r:r + 1, h, 1, :], in_=orow[:])
            # V ones/pad columns
            nc.vector.memset(Vall[:, :, 0, :, 64:65], 32.0)
            nc.vector.memset(Vall[:, :, 1, :, 64:65], 0.0)
            nc.vector.memset(Vall[:, :, 0, :, 65:66], 0.0)
            nc.vector.memset(Vall[:, :, 1, :, 65:66], 0.0)

            negone = res.tile([P, 1], F32, tag="negone", name="negone")
            nc.vector.memset(negone, -1.0)

            # early-fp16 residents
            K16 = res.tile([P, NCG, P], F16, tag="k16", name="K16")
            Q16 = res.tile([P, NCG, P], F16, tag="q16", name="Q16")
            V16 = res.tile([P, HLOC, 65], F16, tag="v16", name="V16")
            nc.vector.memset(V16[:, :, 64:65], 32.0)

            PASSES = [("w", "x"), ("dw", "x"), ("w", "dx")]

            def proj_chains(ps, wname, tts):
                # out [128 toks, 512 cols]: lhsT = x chunk, rhs = W; 256-col halves
                for half in range(2):
                    cs = slice(half * 256, (half + 1) * 256)
                    first = True
                    for pi, (wp, xp) in enumerate(PASSES):
                        wt = w8[("d" if wp == "dw" else "") + wname]
                        xt = dx8s if xp == "dx" else x8s
                        for c in range(4):
                            nc.tensor.matmul(
                                ps[:, cs],
                                xt[:, c, :, tts],
                                wt[:, c, :, cs],
                                start=first,
                                stop=(pi == 2 and c == 3),
                                perf_mode=DR,
                            )
                            first = False

            def emit_attention(tt, tts):
                ntile = tt + 1
                ngrp = (ntile + 3) // 4
                units = [(h, g) for h in range(HLOC) for g in range(ngrp)]
                LAG = 2
                pend = []          # (h, g, sc, nj) awaiting convert
                ready = {}         # (h, g) -> (ex, nj)
                accs = {}

                def do_qk(h, g):
                    cg = h // 2
                    nj = min(4, ntile - 4 * g)
                    sc = psc.tile([P, 4, P], F32, tag="sc", name="sc")
                    for j in range(nj):
                        kt = 4 * g + j
                        nc.tensor.matmul(
                            sc[:, j, :],
                            KT8[:, cg, :, kt * P:(kt + 1) * P],
                            QT8[:, h, :, tts],
                            start=True, stop=(kt != tt), perf_mode=DR,
                        )
                        if kt == tt:
                            nc.tensor.matmul(
                                sc[:, j, :], ramps[:, 0, :, :],
                                ramps[:, 1, :, :],
                                start=False, stop=True, perf_mode=DR,
                            )
                    pend.append((h, g, sc, nj))

                def do_convert():
                    h, g, sc, nj = pend.pop(0)
                    ex = exp8.tile([P, 4, P], U8, tag="ex", name="ex")
                    if (h + tt) % 2 == 0:
                        nc.vector.tensor_scalar(ex[:, 0:nj, :], sc[:, 0:nj, :],
                                                MCONV, 56.0, MULT, ADD)
                    else:
                        nc.scalar.activation(ex[:, 0:nj, :].bitcast(F8),
                                             sc[:, 0:nj, :], EXP,
                                             scale=ACTSC, bias=negone[:, :])
                    ready[(h, g)] = (ex, nj)

                def do_av(h, g):
                    ex, nj = ready.pop((h, g))
                    if g == 0:
                        accs[h] = pacc.tile([66, P], F32, tag="acc", name="acc")
                    acc = accs[h]
                    for j in range(nj):
                        kt = 4 * g + j
                        exd = ex[:, j, :].bitcast(F8).rearrange(
                            "p (one n) -> p one n", one=1
                        ).broadcast_to([P, 2, P])
                        nc.tensor.matmul(
                            acc, Vall[:, kt, :, h, 0:66], exd,
                            start=(kt == 0), stop=(kt == tt),
                            perf_mode=DR,
                        )
                    if 4 * g + nj == ntile:      # head complete
                        cg = h // 2
                        band0 = 0 if h % 2 == 0 else 64
                        rec = work.tile([1, P], F32, tag="rec", name="rec")
                        nc.vector.reciprocal(rec, acc[64:65, :])
                        bc = work.tile([64, P], F32, tag="bc", name="bc")
                        nc.gpsimd.partition_broadcast(bc, rec)
                        nc.vector.tensor_mul(
                            ctx16[band0:band0 + 64, cg, tts], acc[0:64, :], bc)

                for i, (h, g) in enumerate(units):
                    do_qk(h, g)
                    if len(pend) > 1:
                        do_convert()
                    if i >= LAG:
                        do_av(*units[i - LAG])
                while pend:
                    do_convert()
                n = len(units)
                for i in range(max(0, n - LAG), n):
                    do_av(*units[i])

            def emit_attention_early(tts):
                for h in range(HLOC):
                    cg = h // 2
                    band0 = 0 if h % 2 == 0 else 64
                    sc = psc.tile([P, 4, P], F32, tag="sc", name="sc")
                    nc.tensor.matmul(
                        sc[:, 0, :],
                        K16[band0:band0 + 64, cg, :],
                        Q16[band0:band0 + 64, cg, :],
                        start=True, stop=True,
                    )
                    exm = work.tile([P, P], F16, tag="exm", name="exm")
                    nc.scalar.activation(exm, sc[:, 0, :], EXP, scale=ESC)
                    nc.vector.tensor_mul(exm, exm, m16)
                    acc = pacc.tile([66, P], F32, tag="acc", name="acc")
                    nc.tensor.matmul(acc[0:65, :], V16[:, h, :], exm,
                                     start=True, stop=True)
                    rec = work.tile([1, P], F32, tag="rec", name="rec")
                    nc.vector.reciprocal(rec, acc[64:65, :])
                    bc = work.tile([64, P], F32, tag="bc", name="bc")
                    nc.gpsimd.partition_broadcast(bc, rec)
                    nc.vector.tensor_mul(
                        ctx16[band0:band0 + 64, cg, tts], acc[0:64, :], bc)

            def emit_outproj(tt, tts):
                for half in range(2):
                    ps = pout.tile([P, 512], F32, tag="po", name="po")
                    for ct in range(4):
                        nc.tensor.matmul(
                            ps,
                            ctx16[:, ct, tts],
                            wo16[:, ct, half * 512:(half + 1) * 512],
                            start=(ct == 0), stop=(ct == 3),
                        )
                    ob = obp.tile([P, 512], F16, tag="ob", name="ob")
                    nc.scalar.copy(ob, ps)
                    nc.sync.dma_start(
                        out=out[tt * P:(tt + 1) * P,
                                half * 512:(half + 1) * 512],
                        in_=ob,
                    )

            for tt in range(NT):
                tts = slice(tt * P, (tt + 1) * P)
                # ---- V projection ----
                vps = pproj.tile([P, 512], F32, tag="pp", name="vp")
                proj_chains(vps, "wv8", tts)
                vsrc = vps[:, :].rearrange("p (h c) -> p h c", c=64)
                nc.scalar.activation(Vall[:, tt, 0, :, 0:64], vsrc, COPY)
                nc.vector.tensor_sub(
                    Vall[:, tt, 1, :, 0:64], vsrc, Vall[:, tt, 0, :, 0:64])
                # ---- K projection (transposed) ----
                kps_ = pproj.tile([P, 512], F32, tag="pp", name="kp")
                kps = kps_[:, :].rearrange("p (cg n) -> p cg n", cg=4)
                for cg in range(NCG):
                    first = True
                    for pi, (wp, xp) in enumerate(PASSES):
                        wt = w8[("d" if wp == "dw" else "") + "wk8"]
                        xt = dx8s if xp == "dx" else x8s
                        for c in range(4):
                            nc.tensor.matmul(
                                kps[:, cg, :],
                                wt[:, c, :, cg * P:(cg + 1) * P],
                                xt[:, c, :, tts],
                                start=first,
                                stop=(pi == 2 and c == 3),
                                perf_mode=DR,
                            )
                            first = False
                nc.vector.tensor_copy(KT8[:, :, 0, tts], kps)
                nc.vector.tensor_sub(
                    KT8[0:63, :, 1, tts], kps[0:63, :, :], KT8[0:63, :, 0, tts])
                nc.vector.tensor_sub(
                    KT8[64:127, :, 1, tts], kps[64:127, :, :],
                    KT8[64:127, :, 0, tts])
                if tt == 0:
                    nc.scalar.copy(K16, kps)
                # ---- Q projection (transposed) ----
                qps_ = pproj.tile([P, 512], F32, tag="pp", name="qp")
                qps = qps_[:, :].rearrange("p (cg n) -> p cg n", cg=4)
                for cg in range(NCG):
                    first = True
                    for pi, (wp, xp) in enumerate(PASSES):
                        wt = w8[("d" if wp == "dw" else "") + "wq8"]
                        xt = dx8s if xp == "dx" else x8s
                        for c in range(4):
                            nc.tensor.matmul(
                                qps[:, cg, :],
                                wt[:, c, :, cg * P:(cg + 1) * P],
                                xt[:, c, :, tts],
                                start=first,
                                stop=(pi == 2 and c == 3),
                                perf_mode=DR,
                            )
                            first = False
                nc.scalar.activation(QT8[0:64, 0:8:2, 0, tts],
                                     qps[0:64, :, :], COPY, scale=CQ)
                nc.scalar.activation(QT8[0:63, 0:8:2, 1, tts],
                                     qps[0:63, :, :], COPY, scale=CQ)
                nc.scalar.activation(QT8[64:128, 1:8:2, 0, tts],
                                     qps[64:128, :, :], COPY, scale=CQ)
                nc.scalar.activation(QT8[64:127, 1:8:2, 1, tts],
                                     qps[64:127, :, :], COPY, scale=CQ)
                if tt == 0:
                    nc.scalar.copy(Q16, qps)
                    v16src = vps[:, :].rearrange("p (h c) -> p h c", c=64)
                    nc.scalar.copy(V16[:, :, 0:64], v16src)
                # ---- attention for query block tt ----
                if tt == 0:
                    emit_attention_early(tts)
                else:
                    emit_attention(tt, tts)
                emit_outproj(tt, tts)

    return nc


# ---------------------------------------------------------------------------
# Host-side sharding / assembly
# ---------------------------------------------------------------------------

NCORES = 8
B = 4
HS = 32.0  # host weight scale


def _fold_dims(a):
    """[1024 d, N] -> [128, 4, 2, N] with d = p + 128 i + 256 c."""
    n = a.shape[1]
    return np.ascontiguousarray(
        a.reshape(4, 2, 128, n).transpose(2, 0, 1, 3))


def _f8(a):
    return np.asarray(a, np.float32).astype(NP8)


def _shard_inputs(x, Wq, Wk, Wv, Wo, bo):
    x = np.asarray(x, np.float32)
    Wq, Wk, Wv, Wo = (np.asarray(w, np.float32) for w in (Wq, Wk, Wv, Wo))

    qpos = np.arange(S)
    o_bits = A_SCH * np.sqrt(2.0 * np.log(qpos + 1.0))
    orow = _f8(o_bits / (8.0 * MCONV))[None, :]
    krow = np.full((1, S), -8.0, np.float32).astype(NP8)
    zrows = np.zeros((64, 2, S), NP8)
    jj = np.arange(256)
    kk = np.arange(128)
    rampA = (16.0 * (kk[None, :] >= jj[:, None] + 1)).astype(np.float32)
    rampB = (-224.0 * (jj[:, None] >= kk[None, :])).astype(np.float32)
    rampA8 = _f8(rampA.reshape(2, 128, 128).transpose(1, 0, 2))
    rampB8 = _f8(rampB.reshape(2, 128, 128).transpose(1, 0, 2))
    mask16 = (kk[:, None] <= kk[None, :]).astype(np.float16)  # keep k <= q

    xb8, dxb8 = [], []
    for b in range(B):
        xT = np.ascontiguousarray(x[b].T)        # [D, S]
        x8 = _f8(xT)
        dx8 = _f8(xT - x8.astype(np.float32))
        xb8.append(_fold_dims(x8))
        dxb8.append(_fold_dims(dx8))

    wmaps = []
    for g in range(2):
        cols = slice(512 * g, 512 * (g + 1))
        m = {}
        for nm, W in [("wq8", Wq), ("wk8", Wk), ("wv8", Wv)]:
            Ws = (W[:, cols] * HS).astype(np.float32)
            W8 = _f8(Ws)
            dW8 = _f8(Ws - W8.astype(np.float32))
            m[nm] = _fold_dims(W8)
            m["d" + nm] = _fold_dims(dW8)
        wo = Wo[cols, :].astype(np.float16)      # [512, 1024]
        m["wo16"] = np.ascontiguousarray(
            wo.reshape(4, 128, D).transpose(1, 0, 2))
        wmaps.append(m)

    in_maps = []
    for c in range(NCORES):
        b, g = c // 2, c % 2
        in_maps.append({
            "x8f": xb8[b], "dx8f": dxb8[b],
            **wmaps[g],
            "krow": krow, "orow": orow, "zrows": zrows,
            "rampA": rampA8, "rampB": rampB8, "mask16": mask16,
        })
    return in_maps


_NC_CACHE = {}


def _get_program():
    if "nc" not in _NC_CACHE:
        nc = build_program(num_devices=NCORES)
        nc.compile()
        _NC_CACHE["nc"] = nc
    return _NC_CACHE["nc"]


def kernel(x, Wq, Wk, Wv, Wo, bo):
    """Full-input, full-output causal MHA on 8 NeuronCores."""
    from concourse.bass_utils import run_bass_kernel_spmd

    nc = _get_program()
    in_maps = _shard_inputs(x, Wq, Wk, Wv, Wo, bo)
    res = run_bass_kernel_spmd(nc, in_maps, list(range(NCORES)))
    bo32 = np.asarray(bo, np.float32)
    out = np.zeros((B, S, D), np.float32)
    for b in range(B):
        out[b] = (res.results[2 * b]["out"].astype(np.float32)
                  + res.results[2 * b + 1]["out"].astype(np.float32)
                  + bo32[None, :])
    return out


# revision 10
# speedup vs baseline: 1.0291x; 1.0291x over previous
"""8-core Trainium2 Bass kernel for causal MHA — fp8 DoubleRow edition.

Sharding: core c = (batch c//2, head-half c%2); each core computes 8 heads over
the full 2048-token sequence and a partial output projection (contraction over
its 512 local columns); the host sums the two fp16 partials per batch + bias.

Numerics:
- Host scales all of Wq/Wk/Wv by 32 so fp8 residuals stay above e4m3's
  subnormal floor; projections run 3 DoubleRow passes (x8*W8, x8*dW8, dx8*W8)
  with fp32 PSUM accumulation.
- K stored as (K8, dK8) in the DoubleRow fold -> requant-exact scores;
  Q requantized at sigma~1.2 via a scaled copy. Slot-1 row 63/127 of K holds
  -8 and of Q holds o(q)*3.327: the matmul itself subtracts the per-query
  range offset o(q) = 11.54*sqrt(2 ln(q+1)).
- exp via the Schraudolph bit trick: one tensor_scalar (mult M, add 56) to
  uint8 on DVE/Pool, bitcast to fp8e4m3; causal masking inside the diagonal
  block via an extra fp8 "ramp" matmul accumulated into the same PSUM.
- AV in DoubleRow with the fold carrying (V8, dV8); ones-column 32.0 yields
  the softmax normalizer; context normalized to fp16.
- Query block 0 (tokens 0-127) runs a full fp16 path (its rows see no
  softmax averaging, so fp8 noise would land unattenuated).
"""

import sys
import numpy as np

if "/opt/trn_rl_repo" not in sys.path:
    sys.path.insert(0, "/opt/trn_rl_repo")

import ml_dtypes
import concourse.bass as bass
import concourse.tile as tile
from concourse import bacc, mybir

F32 = mybir.dt.float32
F16 = mybir.dt.float16
F8 = mybir.dt.float8e4
U8 = mybir.dt.uint8
EXP = mybir.ActivationFunctionType.Exp
COPY = mybir.ActivationFunctionType.Copy
DR = mybir.MatmulPerfMode.DoubleRow
ADD = mybir.AluOpType.add
MULT = mybir.AluOpType.mult
SUB = mybir.AluOpType.subtract

P = 128
S, D, HLOC, HD = 2048, 1024, 8, 64
NT = S // P                # 16 token tiles
NCG = 4                    # col groups (head pairs) per core
A_SCH = 8 * np.log2(np.e)  # 11.5416
CQ = 1.2 / 32.0            # Q8 = 1.2 q
MCONV = float(A_SCH / 8.0 / (1.2 * 32.0))   # 0.0375696
ESC = 1.0 / 8192.0         # early fp16: psum16 = 8192 s
ACTSC = float(MCONV / 11.5416023862437)   # ACT exp scale: psum*ACTSC = s - o
NP8 = ml_dtypes.float8_e4m3


def build_program(num_devices: int = 8, replicate: int = 1) -> bass.Bass:
    nc = bacc.Bacc("TRN2", target_bir_lowering=False, debug=False,
                   num_devices=num_devices)
    x8f = nc.dram_tensor("x8f", [P, 4, 2, S], F8, kind="ExternalInput")
    dx8f = nc.dram_tensor("dx8f", [P, 4, 2, S], F8, kind="ExternalInput")
    wq8 = nc.dram_tensor("wq8", [P, 4, 2, 512], F8, kind="ExternalInput")
    wk8 = nc.dram_tensor("wk8", [P, 4, 2, 512], F8, kind="ExternalInput")
    wv8 = nc.dram_tensor("wv8", [P, 4, 2, 512], F8, kind="ExternalInput")
    dwq8 = nc.dram_tensor("dwq8", [P, 4, 2, 512], F8, kind="ExternalInput")
    dwk8 = nc.dram_tensor("dwk8", [P, 4, 2, 512], F8, kind="ExternalInput")
    dwv8 = nc.dram_tensor("dwv8", [P, 4, 2, 512], F8, kind="ExternalInput")
    wo16d = nc.dram_tensor("wo16", [P, 4, D], F16, kind="ExternalInput")
    krow = nc.dram_tensor("krow", [1, S], F8, kind="ExternalInput")    # -8
    orow = nc.dram_tensor("orow", [1, S], F8, kind="ExternalInput")    # o*3.327
    zrows = nc.dram_tensor("zrows", [64, 2, S], F8, kind="ExternalInput")
    rampA = nc.dram_tensor("rampA", [P, 2, P], F8, kind="ExternalInput")
    rampB = nc.dram_tensor("rampB", [P, 2, P], F8, kind="ExternalInput")
    mask16 = nc.dram_tensor("mask16", [P, P], F16, kind="ExternalInput")
    out = nc.dram_tensor("out", [S, D], F16, kind="ExternalOutput")

    with tile.TileContext(nc) as tc:
        with (
            tc.tile_pool(name="res", bufs=1) as res,
            tc.tile_pool(name="work", bufs=4) as work,
            tc.tile_pool(name="exp8", bufs=6) as exp8,
            tc.tile_pool(name="obp", bufs=2) as obp,
            tc.tile_pool(name="pproj", bufs=2, space="PSUM") as pproj,
            tc.tile_pool(name="psc", bufs=3, space="PSUM") as psc,
            tc.tile_pool(name="pacc", bufs=2, space="PSUM") as pacc,
            tc.tile_pool(name="pout", bufs=1, space="PSUM") as pout,
        ):
          for _rep in range(replicate):
            # ---------- resident tensors ----------
            KT8 = res.tile([P, NCG, 2, S], F8, tag="kt8", name="KT8")
            QT8 = res.tile([P, HLOC, 2, S], F8, tag="qt8", name="QT8")
            Vall = res.tile([P, NT, 2, HLOC, 80], F8, tag="vall", name="Vall")
            ctx16 = res.tile([P, 4, S], F16, tag="ctx16", name="ctx16")
            x8s = res.tile([P, 4, 2, S], F8, tag="x8s", name="x8s")
            dx8s = res.tile([P, 4, 2, S], F8, tag="dx8s", name="dx8s")
            w8 = {}
            for nm, dt_ in [("wq8", wq8), ("wk8", wk8), ("wv8", wv8),
                            ("dwq8", dwq8), ("dwk8", dwk8), ("dwv8", dwv8)]:
                w8[nm] = res.tile([P, 4, 2, 512], F8, tag=nm, name=nm + "_s")
                nc.sync.dma_start(out=w8[nm], in_=dt_[:])
            wo16 = res.tile([P, 4, D], F16, tag="wo16", name="wo16_s")
            nc.sync.dma_start(out=wo16, in_=wo16d[:])
            nc.sync.dma_start(out=x8s, in_=x8f[:])
            nc.sync.dma_start(out=dx8s, in_=dx8f[:])
            ramps = res.tile([P, 2, 2, P], F8, tag="ramps", name="ramps")
            nc.sync.dma_start(out=ramps[:, 0, :, :], in_=rampA[:])
            nc.sync.dma_start(out=ramps[:, 1, :, :], in_=rampB[:])
            m16 = res.tile([P, P], F16, tag="m16", name="m16")
            nc.sync.dma_start(out=m16, in_=mask16[:])
            # zero bands of QT8 (head h: other-head half zeroed), via zeros DMA
            for h in range(HLOC):
                if h % 2 == 0:
                    nc.sync.dma_start(out=QT8[64:128, h, :, :], in_=zrows[:])
                else:
                    nc.sync.dma_start(out=QT8[0:64, h, :, :], in_=zrows[:])
            # K slot1 const rows (-8) and Q slot1 o-rows
            for cg in range(NCG):
                nc.sync.dma_start(out=KT8[63:64, cg, 1, :], in_=krow[:])
                nc.sync.dma_start(out=KT8[127:128, cg, 1, :], in_=krow[:])
            for h in range(HLOC):
                r = 63 if h % 2 == 0 else 127
                nc.sync.dma_start(out=QT8[r:r + 1, h, 1, :], in_=orow[:])
            # V ones/pad columns
            nc.vector.memset(Vall[:, :, 0, :, 64:65], 32.0)
            nc.vector.memset(Vall[:, :, 1, :, 64:65], 0.0)
            nc.vector.memset(Vall[:, :, 0, :, 65:66], 0.0)
            nc.vector.memset(Vall[:, :, 1, :, 65:66], 0.0)

            negone = res.tile([P, 1], F32, tag="negone", name="negone")
            nc.vector.memset(negone, -1.0)

            # early-fp16 residents
            K16 = res.tile([P, NCG, P], F16, tag="k16", name="K16")
            Q16 = res.tile([P, NCG, P], F16, tag="q16", name="Q16")
            V16 = res.tile([P, HLOC, 65], F16, tag="v16", name="V16")
            nc.vector.memset(V16[:, :, 64:65], 32.0)

            PASSES = [("w", "x"), ("dw", "x"), ("w", "dx")]

            def proj_chains(ps, wname, tts):
                # out [128 toks, 512 cols]: lhsT = x chunk, rhs = W; 256-col halves
                for half in range(2):
                    cs = slice(half * 256, (half + 1) * 256)
                    first = True
                    for pi, (wp, xp) in enumerate(PASSES):
                        wt = w8[("d" if wp == "dw" else "") + wname]
                        xt = dx8s if xp == "dx" else x8s
                        for c in range(4):
                            nc.tensor.matmul(
                                ps[:, cs],
                                xt[:, c, :, tts],
                                wt[:, c, :, cs],
                                start=first,
                                stop=(pi == 2 and c == 3),
                                perf_mode=DR,
                            )
                            first = False

            def emit_attention(tt, tts):
                ntile = tt + 1
                ngrp = (ntile + 3) // 4
                units = [(h, g) for h in range(HLOC) for g in range(ngrp)]
                LAG = 4
                pend = []          # (h, g, sc, nj) awaiting convert
                ready = {}         # (h, g) -> (ex, nj)
                accs = {}

                def do_qk(h, g):
                    cg = h // 2
                    nj = min(4, ntile - 4 * g)
                    sc = psc.tile([P, 4, P], F32, tag="sc", name="sc")
                    for j in range(nj):
                        kt = 4 * g + j
                        nc.tensor.matmul(
                            sc[:, j, :],
                            KT8[:, cg, :, kt * P:(kt + 1) * P],
                            QT8[:, h, :, tts],
                            start=True, stop=(kt != tt), perf_mode=DR,
                        )
                        if kt == tt:
                            nc.tensor.matmul(
                                sc[:, j, :], ramps[:, 0, :, :],
                                ramps[:, 1, :, :],
                                start=False, stop=True, perf_mode=DR,
                            )
                    pend.append((h, g, sc, nj))

                def do_convert():
                    h, g, sc, nj = pend.pop(0)
                    ex = exp8.tile([P, 4, P], U8, tag="ex", name="ex")
                    if (h + tt) % 2 == 0:
                        nc.vector.tensor_scalar(ex[:, 0:nj, :], sc[:, 0:nj, :],
                                                MCONV, 56.0, MULT, ADD)
                    else:
                        nc.scalar.activation(ex[:, 0:nj, :].bitcast(F8),
                                             sc[:, 0:nj, :], EXP,
                                             scale=ACTSC, bias=negone[:, :])
                    ready[(h, g)] = (ex, nj)

                def do_av(h, g):
                    ex, nj = ready.pop((h, g))
                    if g == 0:
                        accs[h] = pacc.tile([66, P], F32, tag="acc", name="acc")
                    acc = accs[h]
                    for j in range(nj):
                        kt = 4 * g + j
                        exd = ex[:, j, :].bitcast(F8).rearrange(
                            "p (one n) -> p one n", one=1
                        ).broadcast_to([P, 2, P])
                        nc.tensor.matmul(
                            acc, Vall[:, kt, :, h, 0:66], exd,
                            start=(kt == 0), stop=(kt == tt),
                            perf_mode=DR,
                        )
                    if 4 * g + nj == ntile:      # head complete
                        cg = h // 2
                        band0 = 0 if h % 2 == 0 else 64
                        rec = work.tile([1, P], F32, tag="rec", name="rec")
                        nc.vector.reciprocal(rec, acc[64:65, :])
                        bc = work.tile([64, P], F32, tag="bc", name="bc")
                        nc.gpsimd.partition_broadcast(bc, rec)
                        nc.vector.tensor_mul(
                            ctx16[band0:band0 + 64, cg, tts], acc[0:64, :], bc)

                for i, (h, g) in enumerate(units):
                    do_qk(h, g)
                    if len(pend) > 1:
                        do_convert()
                    if i >= LAG:
                        do_av(*units[i - LAG])
                while pend:
                    do_convert()
                n = len(units)
                for i in range(max(0, n - LAG), n):
                    do_av(*units[i])

            def emit_attention_early(tts):
                for h in range(HLOC):
                    cg = h // 2
                    band0 = 0 if h % 2 == 0 else 64
                    sc = psc.tile([P, 4, P], F32, tag="sc", name="sc")
                    nc.tensor.matmul(
                        sc[:, 0, :],
                        K16[band0:band0 + 64, cg, :],
                        Q16[band0:band0 + 64, cg, :],
                        start=True, stop=True,
                    )
                    exm = work.tile([P, P], F16, tag="exm", name="exm")
                    nc.scalar.activation(exm, sc[:, 0, :], EXP, scale=ESC)
                    nc.vector.tensor_mul(exm, exm, m16)
                    acc = pacc.tile([66, P], F32, tag="acc", name="acc")
                    nc.tensor.matmul(acc[0:65, :], V16[:, h, :], exm,
                                     start=True, stop=True)
                    rec = work.tile([1, P], F32, tag="rec", name="rec")
                    nc.vector.reciprocal(rec, acc[64:65, :])
                    bc = work.tile([64, P], F32, tag="bc", name="bc")
                    nc.gpsimd.partition_broadcast(bc, rec)
                    nc.vector.tensor_mul(
                        ctx16[band0:band0 + 64, cg, tts], acc[0:64, :], bc)

            def emit_outproj(tt, tts):
                for half in range(2):
                    ps = pout.tile([P, 512], F32, tag="po", name="po")
                    for ct in range(4):
                        nc.tensor.matmul(
                            ps,
                            ctx16[:, ct, tts],
                            wo16[:, ct, half * 512:(half + 1) * 512],
                            start=(ct == 0), stop=(ct == 3),
                        )
                    ob = obp.tile([P, 512], F16, tag="ob", name="ob")
                    nc.scalar.copy(ob, ps)
                    nc.sync.dma_start(
                        out=out[tt * P:(tt + 1) * P,
                                half * 512:(half + 1) * 512],
                        in_=ob,
                    )

            for tt in range(NT):
                tts = slice(tt * P, (tt + 1) * P)
                # ---- V projection ----
                vps = pproj.tile([P, 512], F32, tag="pp", name="vp")
                proj_chains(vps, "wv8", tts)
                vsrc = vps[:, :].rearrange("p (h c) -> p h c", c=64)
                nc.scalar.activation(Vall[:, tt, 0, :, 0:64], vsrc, COPY)
                nc.vector.tensor_sub(
                    Vall[:, tt, 1, :, 0:64], vsrc, Vall[:, tt, 0, :, 0:64])
                # ---- K projection (transposed) ----
                kps_ = pproj.tile([P, 512], F32, tag="pp", name="kp")
                kps = kps_[:, :].rearrange("p (cg n) -> p cg n", cg=4)
                for cg in range(NCG):
                    first = True
                    for pi, (wp, xp) in enumerate(PASSES):
                        wt = w8[("d" if wp == "dw" else "") + "wk8"]
                        xt = dx8s if xp == "dx" else x8s
                        for c in range(4):
                            nc.tensor.matmul(
                                kps[:, cg, :],
                                wt[:, c, :, cg * P:(cg + 1) * P],
                                xt[:, c, :, tts],
                                start=first,
                                stop=(pi == 2 and c == 3),
                                perf_mode=DR,
                            )
                            first = False
                nc.vector.tensor_copy(KT8[:, :, 0, tts], kps)
                nc.vector.tensor_sub(
                    KT8[0:63, :, 1, tts], kps[0:63, :, :], KT8[0:63, :, 0, tts])
                nc.vector.tensor_sub(
                    KT8[64:127, :, 1, tts], kps[64:127, :, :],
                    KT8[64:127, :, 0, tts])
                if tt == 0:
                    nc.scalar.copy(K16, kps)
                # ---- Q projection (transposed) ----
                qps_ = pproj.tile([P, 512], F32, tag="pp", name="qp")
                qps = qps_[:, :].rearrange("p (cg n) -> p cg n", cg=4)
                for cg in range(NCG):
                    first = True
                    for pi, (wp, xp) in enumerate(PASSES):
                        wt = w8[("d" if wp == "dw" else "") + "wq8"]
                        xt = dx8s if xp == "dx" else x8s
                        for c in range(4):
                            nc.tensor.matmul(
                                qps[:, cg, :],
                                wt[:, c, :, cg * P:(cg + 1) * P],
                                xt[:, c, :, tts],
                                start=first,
                                stop=(pi == 2 and c == 3),
                                perf_mode=DR,
                            )
                            first = False
                nc.scalar.activation(QT8[0:64, 0:8:2, 0, tts],
                                     qps[0:64, :, :], COPY, scale=CQ)
                nc.scalar.activation(QT8[0:63, 0:8:2, 1, tts],
                                     qps[0:63, :, :], COPY, scale=CQ)
                nc.scalar.activation(QT8[64:128, 1:8:2, 0, tts],
                                     qps[64:128, :, :], COPY, scale=CQ)
                nc.scalar.activation(QT8[64:127, 1:8:2, 1, tts],
                                     qps[64:127, :, :], COPY, scale=CQ)
                if tt == 0:
                    nc.scalar.copy(Q16, qps)
                    v16src = vps[:, :].rearrange("p (h c) -> p h c", c=64)
                    nc.scalar.copy(V16[:, :, 0:64], v16src)
                # ---- attention for query block tt ----
                if tt == 0:
                    emit_attention_early(tts)
                else:
                    emit_attention(tt, tts)
                emit_outproj(tt, tts)

    return nc


# ---------------------------------------------------------------------------
# Host-side sharding / assembly
# ---------------------------------------------------------------------------

NCORES = 8
B = 4
HS = 32.0  # host weight scale


def _fold_dims(a):
    """[1024 d, N] -> [128, 4, 2, N] with d = p + 128 i + 256 c."""
    n = a.shape[1]
    return np.ascontiguousarray(
        a.reshape(4, 2, 128, n).transpose(2, 0, 1, 3))


def _f8(a):
    return np.asarray(a, np.float32).astype(NP8)


def _shard_inputs(x, Wq, Wk, Wv, Wo, bo):
    x = np.asarray(x, np.float32)
    Wq, Wk, Wv, Wo = (np.asarray(w, np.float32) for w in (Wq, Wk, Wv, Wo))

    qpos = np.arange(S)
    o_bits = A_SCH * np.sqrt(2.0 * np.log(qpos + 1.0))
    orow = _f8(o_bits / (8.0 * MCONV))[None, :]
    krow = np.full((1, S), -8.0, np.float32).astype(NP8)
    zrows = np.zeros((64, 2, S), NP8)
    jj = np.arange(256)
    kk = np.arange(128)
    rampA = (16.0 * (kk[None, :] >= jj[:, None] + 1)).astype(np.float32)
    rampB = (-224.0 * (jj[:, None] >= kk[None, :])).astype(np.float32)
    rampA8 = _f8(rampA.reshape(2, 128, 128).transpose(1, 0, 2))
    rampB8 = _f8(rampB.reshape(2, 128, 128).transpose(1, 0, 2))
    mask16 = (kk[:, None] <= kk[None, :]).astype(np.float16)  # keep k <= q

    xb8, dxb8 = [], []
    for b in range(B):
        xT = np.ascontiguousarray(x[b].T)        # [D, S]
        x8 = _f8(xT)
        dx8 = _f8(xT - x8.astype(np.float32))
        xb8.append(_fold_dims(x8))
        dxb8.append(_fold_dims(dx8))

    wmaps = []
    for g in range(2):
        cols = slice(512 * g, 512 * (g + 1))
        m = {}
        for nm, W in [("wq8", Wq), ("wk8", Wk), ("wv8", Wv)]:
            Ws = (W[:, cols] * HS).astype(np.float32)
            W8 = _f8(Ws)
            dW8 = _f8(Ws - W8.astype(np.float32))
            m[nm] = _fold_dims(W8)
            m["d" + nm] = _fold_dims(dW8)
        wo = Wo[cols, :].astype(np.float16)      # [512, 1024]
        m["wo16"] = np.ascontiguousarray(
            wo.reshape(4, 128, D).transpose(1, 0, 2))
        wmaps.append(m)

    in_maps = []
    for c in range(NCORES):
        b, g = c // 2, c % 2
        in_maps.append({
            "x8f": xb8[b], "dx8f": dxb8[b],
            **wmaps[g],
            "krow": krow, "orow": orow, "zrows": zrows,
            "rampA": rampA8, "rampB": rampB8, "mask16": mask16,
        })
    return in_maps


_NC_CACHE = {}


def _get_program():
    if "nc" not in _NC_CACHE:
        nc = build_program(num_devices=NCORES)
        nc.compile()
        _NC_CACHE["nc"] = nc
    return _NC_CACHE["nc"]


def kernel(x, Wq, Wk, Wv, Wo, bo):
    """Full-input, full-output causal MHA on 8 NeuronCores."""
    from concourse.bass_utils import run_bass_kernel_spmd

    nc = _get_program()
    in_maps = _shard_inputs(x, Wq, Wk, Wv, Wo, bo)
    res = run_bass_kernel_spmd(nc, in_maps, list(range(NCORES)))
    bo32 = np.asarray(bo, np.float32)
    out = np.zeros((B, S, D), np.float32)
    for b in range(B):
        out[b] = (res.results[2 * b]["out"].astype(np.float32)
                  + res.results[2 * b + 1]["out"].astype(np.float32)
                  + bo32[None, :])
    return out


# revision 15
# speedup vs baseline: 1.0475x; 1.0178x over previous
"""8-core Trainium2 Bass kernel for causal MHA — fp8 DoubleRow edition.

Sharding: core c = (batch c//2, head-half c%2); each core computes 8 heads over
the full 2048-token sequence and a partial output projection (contraction over
its 512 local columns); the host sums the two fp16 partials per batch + bias.

Numerics:
- Host scales all of Wq/Wk/Wv by 32 so fp8 residuals stay above e4m3's
  subnormal floor; projections run 3 DoubleRow passes (x8*W8, x8*dW8, dx8*W8)
  with fp32 PSUM accumulation.
- K stored as (K8, dK8) in the DoubleRow fold -> requant-exact scores;
  Q requantized at sigma~1.2 via a scaled copy. Slot-1 row 63/127 of K holds
  -8 and of Q holds o(q)*3.327: the matmul itself subtracts the per-query
  range offset o(q) = 11.54*sqrt(2 ln(q+1)).
- exp via the Schraudolph bit trick: one tensor_scalar (mult M, add 56) to
  uint8 on DVE/Pool, bitcast to fp8e4m3; causal masking inside the diagonal
  block via an extra fp8 "ramp" matmul accumulated into the same PSUM.
- AV in DoubleRow with the fold carrying (V8, dV8); ones-column 32.0 yields
  the softmax normalizer; context normalized to fp16.
- Query block 0 (tokens 0-127) runs a full fp16 path (its rows see no
  softmax averaging, so fp8 noise would land unattenuated).
"""

import sys
import numpy as np

if "/opt/trn_rl_repo" not in sys.path:
    sys.path.insert(0, "/opt/trn_rl_repo")

import ml_dtypes
import concourse.bass as bass
import concourse.tile as tile
from concourse import bacc, mybir

F32 = mybir.dt.float32
F16 = mybir.dt.float16
F8 = mybir.dt.float8e4
U8 = mybir.dt.uint8
EXP = mybir.ActivationFunctionType.Exp
COPY = mybir.ActivationFunctionType.Copy
DR = mybir.MatmulPerfMode.DoubleRow
ADD = mybir.AluOpType.add
MULT = mybir.AluOpType.mult
SUB = mybir.AluOpType.subtract

P = 128
S, D, HLOC, HD = 2048, 1024, 8, 64
NT = S // P                # 16 token tiles
NCG = 4                    # col groups (head pairs) per core
A_SCH = 8 * np.log2(np.e)  # 11.5416
CQ = 1.2 / 32.0            # Q8 = 1.2 q
MCONV = float(A_SCH / 8.0 / (1.2 * 32.0))   # 0.0375696
ESC = 1.0 / 8192.0         # early fp16: psum16 = 8192 s
ACTSC = float(MCONV / 11.5416023862437)   # ACT exp scale: psum*ACTSC = s - o
NP8 = ml_dtypes.float8_e4m3


def build_program(num_devices: int = 8, replicate: int = 1) -> bass.Bass:
    nc = bacc.Bacc("TRN2", target_bir_lowering=False, debug=False,
                   num_devices=num_devices)
    x8f = nc.dram_tensor("x8f", [P, 4, 2, S], F8, kind="ExternalInput")
    dx8f = nc.dram_tensor("dx8f", [P, 4, 2, S], F8, kind="ExternalInput")
    wq8 = nc.dram_tensor("wq8", [P, 4, 2, 512], F8, kind="ExternalInput")
    wk8 = nc.dram_tensor("wk8", [P, 4, 2, 512], F8, kind="ExternalInput")
    wv8 = nc.dram_tensor("wv8", [P, 4, 2, 512], F8, kind="ExternalInput")
    dwq8 = nc.dram_tensor("dwq8", [P, 4, 2, 512], F8, kind="ExternalInput")
    dwk8 = nc.dram_tensor("dwk8", [P, 4, 2, 512], F8, kind="ExternalInput")
    dwv8 = nc.dram_tensor("dwv8", [P, 4, 2, 512], F8, kind="ExternalInput")
    wo16d = nc.dram_tensor("wo16", [P, 4, D], F16, kind="ExternalInput")
    krow = nc.dram_tensor("krow", [1, S], F8, kind="ExternalInput")    # -8
    orow = nc.dram_tensor("orow", [1, S], F8, kind="ExternalInput")    # o*3.327
    zrows = nc.dram_tensor("zrows", [64, 2, S], F8, kind="ExternalInput")
    rampA = nc.dram_tensor("rampA", [P, 2, P], F8, kind="ExternalInput")
    rampB = nc.dram_tensor("rampB", [P, 2, P], F8, kind="ExternalInput")
    mask16 = nc.dram_tensor("mask16", [P, P], F16, kind="ExternalInput")
    out = nc.dram_tensor("out", [S, D], F16, kind="ExternalOutput")

    with tile.TileContext(nc) as tc:
        with (
            tc.tile_pool(name="res", bufs=1) as res,
            tc.tile_pool(name="work", bufs=4) as work,
            tc.tile_pool(name="exp8", bufs=9) as exp8,
            tc.tile_pool(name="obp", bufs=2) as obp,
            tc.tile_pool(name="pproj", bufs=2, space="PSUM") as pproj,
            tc.tile_pool(name="psc", bufs=3, space="PSUM") as psc,
            tc.tile_pool(name="pacc", bufs=2, space="PSUM") as pacc,
            tc.tile_pool(name="pout", bufs=1, space="PSUM") as pout,
        ):
          for _rep in range(replicate):
            # ---------- resident tensors ----------
            KT8 = res.tile([P, NCG, 2, S], F8, tag="kt8", name="KT8")
            QT8 = res.tile([P, HLOC, 2, S], F8, tag="qt8", name="QT8")
            Vall = res.tile([P, NT, 2, HLOC, 80], F8, tag="vall", name="Vall")
            ctx16 = res.tile([P, 4, S], F16, tag="ctx16", name="ctx16")
            x8s = res.tile([P, 4, 2, S], F8, tag="x8s", name="x8s")
            dx8s = res.tile([P, 4, 2, S], F8, tag="dx8s", name="dx8s")
            w8 = {}
            for nm in ["wq8", "wk8", "wv8", "dwq8", "dwk8", "dwv8"]:
                w8[nm] = res.tile([P, 4, 2, 512], F8, tag=nm, name=nm + "_s")
            # DMA order = first-use order: pass-1 weights + x8, then residuals,
            # then everything only needed later (wo16, attention consts).
            for nm, dt_ in [("wv8", wv8), ("wk8", wk8), ("wq8", wq8)]:
                nc.sync.dma_start(out=w8[nm], in_=dt_[:])
            nc.sync.dma_start(out=x8s, in_=x8f[:])
            for nm, dt_ in [("dwv8", dwv8), ("dwk8", dwk8), ("dwq8", dwq8)]:
                nc.sync.dma_start(out=w8[nm], in_=dt_[:])
            nc.sync.dma_start(out=dx8s, in_=dx8f[:])
            wo16 = res.tile([P, 4, D], F16, tag="wo16", name="wo16_s")
            nc.sync.dma_start(out=wo16, in_=wo16d[:])
            ramps = res.tile([P, 2, 2, P], F8, tag="ramps", name="ramps")
            nc.sync.dma_start(out=ramps[:, 0, :, :], in_=rampA[:])
            nc.sync.dma_start(out=ramps[:, 1, :, :], in_=rampB[:])
            m16 = res.tile([P, P], F16, tag="m16", name="m16")
            nc.sync.dma_start(out=m16, in_=mask16[:])
            # zero bands of QT8 (head h: other-head half zeroed), via zeros DMA
            for h in range(HLOC):
                if h % 2 == 0:
                    nc.sync.dma_start(out=QT8[64:128, h, :, :], in_=zrows[:])
                else:
                    nc.sync.dma_start(out=QT8[0:64, h, :, :], in_=zrows[:])
            # K slot1 const rows (-8) and Q slot1 o-rows
            for cg in range(NCG):
                nc.sync.dma_start(out=KT8[63:64, cg, 1, :], in_=krow[:])
                nc.sync.dma_start(out=KT8[127:128, cg, 1, :], in_=krow[:])
            for h in range(HLOC):
                r = 63 if h % 2 == 0 else 127
                nc.sync.dma_start(out=QT8[r:r + 1, h, 1, :], in_=orow[:])
            # V ones/pad columns
            nc.vector.memset(Vall[:, :, 0, :, 64:65], 32.0)
            nc.vector.memset(Vall[:, :, 1, :, 64:65], 0.0)
            nc.vector.memset(Vall[:, :, 0, :, 65:66], 0.0)
            nc.vector.memset(Vall[:, :, 1, :, 65:66], 0.0)

            negone = res.tile([P, 1], F32, tag="negone", name="negone")
            nc.vector.memset(negone, -1.0)

            # early-fp16 residents
            K16 = res.tile([P, NCG, P], F16, tag="k16", name="K16")
            Q16 = res.tile([P, NCG, P], F16, tag="q16", name="Q16")
            V16 = res.tile([P, HLOC, 65], F16, tag="v16", name="V16")
            nc.vector.memset(V16[:, :, 64:65], 32.0)

            PASSES = [("w", "x"), ("dw", "x"), ("w", "dx")]

            def proj_chains(ps, wname, tts):
                # out [128 toks, 512 cols]: lhsT = x chunk, rhs = W; 256-col halves
                for half in range(2):
                    cs = slice(half * 256, (half + 1) * 256)
                    first = True
                    for pi, (wp, xp) in enumerate(PASSES):
                        wt = w8[("d" if wp == "dw" else "") + wname]
                        xt = dx8s if xp == "dx" else x8s
                        for c in range(4):
                            nc.tensor.matmul(
                                ps[:, cs],
                                xt[:, c, :, tts],
                                wt[:, c, :, cs],
                                start=first,
                                stop=(pi == 2 and c == 3),
                                perf_mode=DR,
                            )
                            first = False

            def emit_attention(tt, tts):
                ntile = tt + 1
                ngrp = (ntile + 3) // 4
                units = [(h, g) for h in range(HLOC) for g in range(ngrp)]
                LAG = 6
                pend = []          # (h, g, sc, nj) awaiting convert
                ready = {}         # (h, g) -> (ex, nj)
                accs = {}

                def do_qk(h, g):
                    cg = h // 2
                    nj = min(4, ntile - 4 * g)
                    sc = psc.tile([P, 4, P], F32, tag="sc", name="sc")
                    for j in range(nj):
                        kt = 4 * g + j
                        nc.tensor.matmul(
                            sc[:, j, :],
                            KT8[:, cg, :, kt * P:(kt + 1) * P],
                            QT8[:, h, :, tts],
                            start=True, stop=(kt != tt), perf_mode=DR,
                        )
                        if kt == tt:
                            nc.tensor.matmul(
                                sc[:, j, :], ramps[:, 0, :, :],
                                ramps[:, 1, :, :],
                                start=False, stop=True, perf_mode=DR,
                            )
                    pend.append((h, g, sc, nj))

                def do_convert():
                    h, g, sc, nj = pend.pop(0)
                    ex = exp8.tile([P, 4, P], U8, tag="ex", name="ex")
                    if (h + tt) % 2 == 0:
                        nc.vector.tensor_scalar(ex[:, 0:nj, :], sc[:, 0:nj, :],
                                                MCONV, 56.0, MULT, ADD)
                    else:
                        nc.scalar.activation(ex[:, 0:nj, :].bitcast(F8),
                                             sc[:, 0:nj, :], EXP,
                                             scale=ACTSC, bias=negone[:, :])
                    ready[(h, g)] = (ex, nj)

                def do_av(h, g):
                    ex, nj = ready.pop((h, g))
                    if g == 0:
                        accs[h] = pacc.tile([66, P], F32, tag="acc", name="acc")
                    acc = accs[h]
                    for j in range(nj):
                        kt = 4 * g + j
                        exd = ex[:, j, :].bitcast(F8).rearrange(
                            "p (one n) -> p one n", one=1
                        ).broadcast_to([P, 2, P])
                        nc.tensor.matmul(
                            acc, Vall[:, kt, :, h, 0:66], exd,
                            start=(kt == 0), stop=(kt == tt),
                            perf_mode=DR,
                        )
                    if 4 * g + nj == ntile:      # head complete
                        cg = h // 2
                        band0 = 0 if h % 2 == 0 else 64
                        rec = work.tile([1, P], F32, tag="rec", name="rec")
                        nc.vector.reciprocal(rec, acc[64:65, :])
                        bc = work.tile([64, P], F32, tag="bc", name="bc")
                        nc.gpsimd.partition_broadcast(bc, rec)
                        nc.vector.tensor_mul(
                            ctx16[band0:band0 + 64, cg, tts], acc[0:64, :], bc)

                for i, (h, g) in enumerate(units):
                    do_qk(h, g)
                    if len(pend) > 1:
                        do_convert()
                    if i >= LAG:
                        do_av(*units[i - LAG])
                while pend:
                    do_convert()
                n = len(units)
                for i in range(max(0, n - LAG), n):
                    do_av(*units[i])

            def emit_attention_early(tts):
                for h in range(HLOC):
                    cg = h // 2
                    band0 = 0 if h % 2 == 0 else 64
                    sc = psc.tile([P, 4, P], F32, tag="sc", name="sc")
                    nc.tensor.matmul(
                        sc[:, 0, :],
                        K16[band0:band0 + 64, cg, :],
                        Q16[band0:band0 + 64, cg, :],
                        start=True, stop=True,
                    )
                    exm = work.tile([P, P], F16, tag="exm", name="exm")
                    nc.scalar.activation(exm, sc[:, 0, :], EXP, scale=ESC)
                    nc.vector.tensor_mul(exm, exm, m16)
                    acc = pacc.tile([66, P], F32, tag="acc", name="acc")
                    nc.tensor.matmul(acc[0:65, :], V16[:, h, :], exm,
                                     start=True, stop=True)
                    rec = work.tile([1, P], F32, tag="rec", name="rec")
                    nc.vector.reciprocal(rec, acc[64:65, :])
                    bc = work.tile([64, P], F32, tag="bc", name="bc")
                    nc.gpsimd.partition_broadcast(bc, rec)
                    nc.vector.tensor_mul(
                        ctx16[band0:band0 + 64, cg, tts], acc[0:64, :], bc)

            def emit_outproj(tt, tts):
                for half in range(2):
                    ps = pout.tile([P, 512], F32, tag="po", name="po")
                    for ct in range(4):
                        nc.tensor.matmul(
                            ps,
                            ctx16[:, ct, tts],
                            wo16[:, ct, half * 512:(half + 1) * 512],
                            start=(ct == 0), stop=(ct == 3),
                        )
                    ob = obp.tile([P, 512], F16, tag="ob", name="ob")
                    nc.scalar.copy(ob, ps)
                    nc.sync.dma_start(
                        out=out[tt * P:(tt + 1) * P,
                                half * 512:(half + 1) * 512],
                        in_=ob,
                    )

            for tt in range(NT):
                tts = slice(tt * P, (tt + 1) * P)
                # ---- V projection ----
                vps = pproj.tile([P, 512], F32, tag="pp", name="vp")
                proj_chains(vps, "wv8", tts)
                vsrc = vps[:, :].rearrange("p (h c) -> p h c", c=64)
                nc.scalar.activation(Vall[:, tt, 0, :, 0:64], vsrc, COPY)
                nc.vector.tensor_sub(
                    Vall[:, tt, 1, :, 0:64], vsrc, Vall[:, tt, 0, :, 0:64])
                # ---- K projection (transposed) ----
                kps_ = pproj.tile([P, 512], F32, tag="pp", name="kp")
                kps = kps_[:, :].rearrange("p (cg n) -> p cg n", cg=4)
                for cg in range(NCG):
                    first = True
                    for pi, (wp, xp) in enumerate(PASSES):
                        wt = w8[("d" if wp == "dw" else "") + "wk8"]
                        xt = dx8s if xp == "dx" else x8s
                        for c in range(4):
                            nc.tensor.matmul(
                                kps[:, cg, :],
                                wt[:, c, :, cg * P:(cg + 1) * P],
                                xt[:, c, :, tts],
                                start=first,
                                stop=(pi == 1 and c == 3),
                                perf_mode=DR,
                            )
                            first = False
                nc.vector.tensor_copy(KT8[:, :, 0, tts], kps)
                nc.vector.tensor_sub(
                    KT8[0:63, :, 1, tts], kps[0:63, :, :], KT8[0:63, :, 0, tts])
                nc.vector.tensor_sub(
                    KT8[64:127, :, 1, tts], kps[64:127, :, :],
                    KT8[64:127, :, 0, tts])
                if tt == 0:
                    nc.scalar.copy(K16, kps)
                # ---- Q projection (transposed) ----
                qps_ = pproj.tile([P, 512], F32, tag="pp", name="qp")
                qps = qps_[:, :].rearrange("p (cg n) -> p cg n", cg=4)
                for cg in range(NCG):
                    first = True
                    for pi, (wp, xp) in enumerate(PASSES):
                        wt = w8[("d" if wp == "dw" else "") + "wq8"]
                        xt = dx8s if xp == "dx" else x8s
                        for c in range(4):
                            nc.tensor.matmul(
                                qps[:, cg, :],
                                wt[:, c, :, cg * P:(cg + 1) * P],
                                xt[:, c, :, tts],
                                start=first,
                                stop=(pi == 1 and c == 3),
                                perf_mode=DR,
                            )
                            first = False
                nc.scalar.activation(QT8[0:64, 0:8:2, 0, tts],
                                     qps[0:64, :, :], COPY, scale=CQ)
                nc.scalar.activation(QT8[0:63, 0:8:2, 1, tts],
                                     qps[0:63, :, :], COPY, scale=CQ)
                nc.scalar.activation(QT8[64:128, 1:8:2, 0, tts],
                                     qps[64:128, :, :], COPY, scale=CQ)
                nc.scalar.activation(QT8[64:127, 1:8:2, 1, tts],
                                     qps[64:127, :, :], COPY, scale=CQ)
                if tt == 0:
                    nc.scalar.copy(Q16, qps)
                    v16src = vps[:, :].rearrange("p (h c) -> p h c", c=64)
                    nc.scalar.copy(V16[:, :, 0:64], v16src)
                # ---- attention for query block tt ----
                if tt == 0:
                    emit_attention_early(tts)
                else:
                    emit_attention(tt, tts)
                emit_outproj(tt, tts)

    return nc


# ---------------------------------------------------------------------------
# Host-side sharding / assembly
# ---------------------------------------------------------------------------

NCORES = 8
B = 4
HS = 32.0  # host weight scale


def _fold_dims(a):
    """[1024 d, N] -> [128, 4, 2, N] with d = p + 128 i + 256 c."""
    n = a.shape[1]
    return np.ascontiguousarray(
        a.reshape(4, 2, 128, n).transpose(2, 0, 1, 3))


def _f8(a):
    return np.asarray(a, np.float32).astype(NP8)


def _shard_inputs(x, Wq, Wk, Wv, Wo, bo):
    x = np.asarray(x, np.float32)
    Wq, Wk, Wv, Wo = (np.asarray(w, np.float32) for w in (Wq, Wk, Wv, Wo))

    qpos = np.arange(S)
    o_bits = A_SCH * np.sqrt(2.0 * np.log(qpos + 1.0))
    orow = _f8(o_bits / (8.0 * MCONV))[None, :]
    krow = np.full((1, S), -8.0, np.float32).astype(NP8)
    zrows = np.zeros((64, 2, S), NP8)
    jj = np.arange(256)
    kk = np.arange(128)
    rampA = (16.0 * (kk[None, :] >= jj[:, None] + 1)).astype(np.float32)
    rampB = (-224.0 * (jj[:, None] >= kk[None, :])).astype(np.float32)
    rampA8 = _f8(rampA.reshape(2, 128, 128).transpose(1, 0, 2))
    rampB8 = _f8(rampB.reshape(2, 128, 128).transpose(1, 0, 2))
    mask16 = (kk[:, None] <= kk[None, :]).astype(np.float16)  # keep k <= q

    xb8, dxb8 = [], []
    for b in range(B):
        xT = np.ascontiguousarray(x[b].T)        # [D, S]
        x8 = _f8(xT)
        dx8 = _f8(xT - x8.astype(np.float32))
        xb8.append(_fold_dims(x8))
        dxb8.append(_fold_dims(dx8))

    wmaps = []
    for g in range(2):
        cols = slice(512 * g, 512 * (g + 1))
        m = {}
        for nm, W in [("wq8", Wq), ("wk8", Wk), ("wv8", Wv)]:
            Ws = (W[:, cols] * HS).astype(np.float32)
            W8 = _f8(Ws)
            dW8 = _f8(Ws - W8.astype(np.float32))
            m[nm] = _fold_dims(W8)
            m["d" + nm] = _fold_dims(dW8)
        wo = Wo[cols, :].astype(np.float16)      # [512, 1024]
        m["wo16"] = np.ascontiguousarray(
            wo.reshape(4, 128, D).transpose(1, 0, 2))
        wmaps.append(m)

    in_maps = []
    for c in range(NCORES):
        b, g = c // 2, c % 2
        in_maps.append({
            "x8f": xb8[b], "dx8f": dxb8[b],
            **wmaps[g],
            "krow": krow, "orow": orow, "zrows": zrows,
            "rampA": rampA8, "rampB": rampB8, "mask16": mask16,
        })
    return in_maps


_NC_CACHE = {}


def _get_program():
    if "nc" not in _NC_CACHE:
        nc = build_program(num_devices=NCORES)
        nc.compile()
        _NC_CACHE["nc"] = nc
    return _NC_CACHE["nc"]


def kernel(x, Wq, Wk, Wv, Wo, bo):
    """Full-input, full-output causal MHA on 8 NeuronCores."""
    from concourse.bass_utils import run_bass_kernel_spmd

    nc = _get_program()
    in_maps = _shard_inputs(x, Wq, Wk, Wv, Wo, bo)
    res = run_bass_kernel_spmd(nc, in_maps, list(range(NCORES)))
    bo32 = np.asarray(bo, np.float32)
    out = np.zeros((B, S, D), np.float32)
    for b in range(B):
        out[b] = (res.results[2 * b]["out"].astype(np.float32)
                  + res.results[2 * b + 1]["out"].astype(np.float32)
                  + bo32[None, :])
    return out


# revision 16
# speedup vs baseline: 1.0505x; 1.0029x over previous
"""8-core Trainium2 Bass kernel for causal MHA — fp8 DoubleRow edition.

Sharding: core c = (batch c//2, head-half c%2); each core computes 8 heads over
the full 2048-token sequence and a partial output projection (contraction over
its 512 local columns); the host sums the two fp16 partials per batch + bias.

Numerics:
- Host scales all of Wq/Wk/Wv by 32 so fp8 residuals stay above e4m3's
  subnormal floor; projections run 3 DoubleRow passes (x8*W8, x8*dW8, dx8*W8)
  with fp32 PSUM accumulation.
- K stored as (K8, dK8) in the DoubleRow fold -> requant-exact scores;
  Q requantized at sigma~1.2 via a scaled copy. Slot-1 row 63/127 of K holds
  -8 and of Q holds o(q)*3.327: the matmul itself subtracts the per-query
  range offset o(q) = 11.54*sqrt(2 ln(q+1)).
- exp via the Schraudolph bit trick: one tensor_scalar (mult M, add 56) to
  uint8 on DVE/Pool, bitcast to fp8e4m3; causal masking inside the diagonal
  block via an extra fp8 "ramp" matmul accumulated into the same PSUM.
- AV in DoubleRow with the fold carrying (V8, dV8); ones-column 32.0 yields
  the softmax normalizer; context normalized to fp16.
- Query block 0 (tokens 0-127) runs a full fp16 path (its rows see no
  softmax averaging, so fp8 noise would land unattenuated).
"""

import sys
import numpy as np

if "/opt/trn_rl_repo" not in sys.path:
    sys.path.insert(0, "/opt/trn_rl_repo")

import ml_dtypes
import concourse.bass as bass
import concourse.tile as tile
from concourse import bacc, mybir

F32 = mybir.dt.float32
F16 = mybir.dt.float16
F8 = mybir.dt.float8e4
U8 = mybir.dt.uint8
EXP = mybir.ActivationFunctionType.Exp
COPY = mybir.ActivationFunctionType.Copy
DR = mybir.MatmulPerfMode.DoubleRow
ADD = mybir.AluOpType.add
MULT = mybir.AluOpType.mult
SUB = mybir.AluOpType.subtract

P = 128
S, D, HLOC, HD = 2048, 1024, 8, 64
NT = S // P                # 16 token tiles
NCG = 4                    # col groups (head pairs) per core
A_SCH = 8 * np.log2(np.e)  # 11.5416
CQ = 1.2 / 32.0            # Q8 = 1.2 q
MCONV = float(A_SCH / 8.0 / (1.2 * 32.0))   # 0.0375696
ESC = 1.0 / 8192.0         # early fp16: psum16 = 8192 s
ACTSC = float(MCONV / 11.5416023862437)   # ACT exp scale: psum*ACTSC = s - o
NP8 = ml_dtypes.float8_e4m3


def build_program(num_devices: int = 8, replicate: int = 1) -> bass.Bass:
    nc = bacc.Bacc("TRN2", target_bir_lowering=False, debug=False,
                   num_devices=num_devices)
    x8f = nc.dram_tensor("x8f", [P, 4, 2, S], F8, kind="ExternalInput")
    dx8f = nc.dram_tensor("dx8f", [P, 4, 2, S], F8, kind="ExternalInput")
    wq8 = nc.dram_tensor("wq8", [P, 4, 2, 512], F8, kind="ExternalInput")
    wk8 = nc.dram_tensor("wk8", [P, 4, 2, 512], F8, kind="ExternalInput")
    wv8 = nc.dram_tensor("wv8", [P, 4, 2, 512], F8, kind="ExternalInput")
    dwq8 = nc.dram_tensor("dwq8", [P, 4, 2, 512], F8, kind="ExternalInput")
    dwk8 = nc.dram_tensor("dwk8", [P, 4, 2, 512], F8, kind="ExternalInput")
    dwv8 = nc.dram_tensor("dwv8", [P, 4, 2, 512], F8, kind="ExternalInput")
    wo16d = nc.dram_tensor("wo16", [P, 4, D], F16, kind="ExternalInput")
    krow = nc.dram_tensor("krow", [1, S], F8, kind="ExternalInput")    # -8
    orow = nc.dram_tensor("orow", [1, S], F8, kind="ExternalInput")    # o*3.327
    zrows = nc.dram_tensor("zrows", [64, 2, S], F8, kind="ExternalInput")
    rampA = nc.dram_tensor("rampA", [P, 2, P], F8, kind="ExternalInput")
    rampB = nc.dram_tensor("rampB", [P, 2, P], F8, kind="ExternalInput")
    mask16 = nc.dram_tensor("mask16", [P, P], F16, kind="ExternalInput")
    out = nc.dram_tensor("out", [S, D], F16, kind="ExternalOutput")

    with tile.TileContext(nc) as tc:
        with (
            tc.tile_pool(name="res", bufs=1) as res,
            tc.tile_pool(name="work", bufs=4) as work,
            tc.tile_pool(name="exp8", bufs=9) as exp8,
            tc.tile_pool(name="obp", bufs=2) as obp,
            tc.tile_pool(name="pproj", bufs=2, space="PSUM") as pproj,
            tc.tile_pool(name="psc", bufs=3, space="PSUM") as psc,
            tc.tile_pool(name="pacc", bufs=2, space="PSUM") as pacc,
            tc.tile_pool(name="pout", bufs=1, space="PSUM") as pout,
        ):
          for _rep in range(replicate):
            # ---------- resident tensors ----------
            KT8 = res.tile([P, NCG, 2, S], F8, tag="kt8", name="KT8")
            QT8 = res.tile([P, HLOC, 2, S], F8, tag="qt8", name="QT8")
            Vall = res.tile([P, NT, 2, HLOC, 80], F8, tag="vall", name="Vall")
            ctx16 = res.tile([P, 4, S], F16, tag="ctx16", name="ctx16")
            x8s = res.tile([P, 4, 2, S], F8, tag="x8s", name="x8s")
            dx8s = res.tile([P, 4, 2, S], F8, tag="dx8s", name="dx8s")
            w8 = {}
            for nm in ["wq8", "wk8", "wv8", "dwq8", "dwk8", "dwv8"]:
                w8[nm] = res.tile([P, 4, 2, 512], F8, tag=nm, name=nm + "_s")
            # DMA order = first-use order: pass-1 weights + x8, then residuals,
            # then everything only needed later (wo16, attention consts).
            for nm, dt_ in [("wv8", wv8), ("wk8", wk8), ("wq8", wq8)]:
                nc.sync.dma_start(out=w8[nm], in_=dt_[:])
            nc.sync.dma_start(out=x8s, in_=x8f[:])
            for nm, dt_ in [("dwv8", dwv8), ("dwk8", dwk8), ("dwq8", dwq8)]:
                nc.sync.dma_start(out=w8[nm], in_=dt_[:])
            nc.sync.dma_start(out=dx8s, in_=dx8f[:])
            wo16 = res.tile([P, 4, D], F16, tag="wo16", name="wo16_s")
            nc.sync.dma_start(out=wo16, in_=wo16d[:])
            ramps = res.tile([P, 2, 2, P], F8, tag="ramps", name="ramps")
            nc.sync.dma_start(out=ramps[:, 0, :, :], in_=rampA[:])
            nc.sync.dma_start(out=ramps[:, 1, :, :], in_=rampB[:])
            m16 = res.tile([P, P], F16, tag="m16", name="m16")
            nc.sync.dma_start(out=m16, in_=mask16[:])
            # zero bands of QT8 (head h: other-head half zeroed), via zeros DMA
            for h in range(HLOC):
                if h % 2 == 0:
                    nc.sync.dma_start(out=QT8[64:128, h, :, :], in_=zrows[:])
                else:
                    nc.sync.dma_start(out=QT8[0:64, h, :, :], in_=zrows[:])
            # K slot1 const rows (-8) and Q slot1 o-rows
            for cg in range(NCG):
                nc.sync.dma_start(out=KT8[63:64, cg, 1, :], in_=krow[:])
                nc.sync.dma_start(out=KT8[127:128, cg, 1, :], in_=krow[:])
            for h in range(HLOC):
                r = 63 if h % 2 == 0 else 127
                nc.sync.dma_start(out=QT8[r:r + 1, h, 1, :], in_=orow[:])
            # V ones/pad columns
            nc.vector.memset(Vall[:, :, 0, :, 64:65], 32.0)
            nc.vector.memset(Vall[:, :, 1, :, 64:65], 0.0)
            nc.vector.memset(Vall[:, :, 0, :, 65:66], 0.0)
            nc.vector.memset(Vall[:, :, 1, :, 65:66], 0.0)

            negone = res.tile([P, 1], F32, tag="negone", name="negone")
            nc.vector.memset(negone, -1.0)

            # early-fp16 residents
            K16 = res.tile([P, NCG, P], F16, tag="k16", name="K16")
            Q16 = res.tile([P, NCG, P], F16, tag="q16", name="Q16")
            V16 = res.tile([P, HLOC, 65], F16, tag="v16", name="V16")
            nc.vector.memset(V16[:, :, 64:65], 32.0)

            PASSES = [("w", "x"), ("dw", "x"), ("w", "dx")]

            def proj_chains(ps, wname, tts):
                # out [128 toks, 512 cols]: lhsT = x chunk, rhs = W; 256-col halves
                for half in range(2):
                    cs = slice(half * 256, (half + 1) * 256)
                    first = True
                    for pi, (wp, xp) in enumerate(PASSES):
                        wt = w8[("d" if wp == "dw" else "") + wname]
                        xt = dx8s if xp == "dx" else x8s
                        for c in range(4):
                            nc.tensor.matmul(
                                ps[:, cs],
                                xt[:, c, :, tts],
                                wt[:, c, :, cs],
                                start=first,
                                stop=(pi == 2 and c == 3),
                                perf_mode=DR,
                            )
                            first = False

            def emit_attention(tt, tts):
                ntile = tt + 1
                ngrp = (ntile + 3) // 4
                units = [(h, g) for h in range(HLOC) for g in range(ngrp)]
                LAG = 6
                pend = []          # (h, g, sc, nj) awaiting convert
                ready = {}         # (h, g) -> (ex, nj)
                accs = {}

                def do_qk(h, g):
                    cg = h // 2
                    nj = min(4, ntile - 4 * g)
                    sc = psc.tile([P, 4, P], F32, tag="sc", name="sc")
                    for j in range(nj):
                        kt = 4 * g + j
                        nc.tensor.matmul(
                            sc[:, j, :],
                            KT8[:, cg, :, kt * P:(kt + 1) * P],
                            QT8[:, h, :, tts],
                            start=True, stop=(kt != tt), perf_mode=DR,
                        )
                        if kt == tt:
                            nc.tensor.matmul(
                                sc[:, j, :], ramps[:, 0, :, :],
                                ramps[:, 1, :, :],
                                start=False, stop=True, perf_mode=DR,
                            )
                    pend.append((h, g, sc, nj))

                def do_convert():
                    h, g, sc, nj = pend.pop(0)
                    ex = exp8.tile([P, 4, P], U8, tag="ex", name="ex")
                    if (h + tt) % 2 == 0:
                        nc.vector.tensor_scalar(ex[:, 0:nj, :], sc[:, 0:nj, :],
                                                MCONV, 56.0, MULT, ADD)
                    else:
                        nc.scalar.activation(ex[:, 0:nj, :].bitcast(F8),
                                             sc[:, 0:nj, :], EXP,
                                             scale=ACTSC, bias=negone[:, :])
                    ready[(h, g)] = (ex, nj)

                def do_av(h, g):
                    ex, nj = ready.pop((h, g))
                    if g == 0:
                        accs[h] = pacc.tile([66, P], F32, tag="acc", name="acc")
                    acc = accs[h]
                    for j in range(nj):
                        kt = 4 * g + j
                        exd = ex[:, j, :].bitcast(F8).rearrange(
                            "p (one n) -> p one n", one=1
                        ).broadcast_to([P, 2, P])
                        nc.tensor.matmul(
                            acc, Vall[:, kt, :, h, 0:66], exd,
                            start=(kt == 0), stop=(kt == tt),
                            perf_mode=DR,
                        )
                    if 4 * g + nj == ntile:      # head complete
                        cg = h // 2
                        band0 = 0 if h % 2 == 0 else 64
                        rec = work.tile([1, P], F32, tag="rec", name="rec")
                        nc.vector.reciprocal(rec, acc[64:65, :])
                        bc = work.tile([64, P], F32, tag="bc", name="bc")
                        nc.gpsimd.partition_broadcast(bc, rec)
                        nc.vector.tensor_mul(
                            ctx16[band0:band0 + 64, cg, tts], acc[0:64, :], bc)

                for i, (h, g) in enumerate(units):
                    do_qk(h, g)
                    while pend:
                        do_convert()
                    if i >= LAG:
                        do_av(*units[i - LAG])
                while pend:
                    do_convert()
                n = len(units)
                for i in range(max(0, n - LAG), n):
                    do_av(*units[i])

            def emit_attention_early(tts):
                for h in range(HLOC):
                    cg = h // 2
                    band0 = 0 if h % 2 == 0 else 64
                    sc = psc.tile([P, 4, P], F32, tag="sc", name="sc")
                    nc.tensor.matmul(
                        sc[:, 0, :],
                        K16[band0:band0 + 64, cg, :],
                        Q16[band0:band0 + 64, cg, :],
                        start=True, stop=True,
                    )
                    exm = work.tile([P, P], F16, tag="exm", name="exm")
                    nc.scalar.activation(exm, sc[:, 0, :], EXP, scale=ESC)
                    nc.vector.tensor_mul(exm, exm, m16)
                    acc = pacc.tile([66, P], F32, tag="acc", name="acc")
                    nc.tensor.matmul(acc[0:65, :], V16[:, h, :], exm,
                                     start=True, stop=True)
                    rec = work.tile([1, P], F32, tag="rec", name="rec")
                    nc.vector.reciprocal(rec, acc[64:65, :])
                    bc = work.tile([64, P], F32, tag="bc", name="bc")
                    nc.gpsimd.partition_broadcast(bc, rec)
                    nc.vector.tensor_mul(
                        ctx16[band0:band0 + 64, cg, tts], acc[0:64, :], bc)

            def emit_outproj(tt, tts):
                for half in range(2):
                    ps = pout.tile([P, 512], F32, tag="po", name="po")
                    for ct in range(4):
                        nc.tensor.matmul(
                            ps,
                            ctx16[:, ct, tts],
                            wo16[:, ct, half * 512:(half + 1) * 512],
                            start=(ct == 0), stop=(ct == 3),
                        )
                    ob = obp.tile([P, 512], F16, tag="ob", name="ob")
                    nc.scalar.copy(ob, ps)
                    nc.sync.dma_start(
                        out=out[tt * P:(tt + 1) * P,
                                half * 512:(half + 1) * 512],
                        in_=ob,
                    )

            for tt in range(NT):
                tts = slice(tt * P, (tt + 1) * P)
                # ---- V projection ----
                vps = pproj.tile([P, 512], F32, tag="pp", name="vp")
                proj_chains(vps, "wv8", tts)
                vsrc = vps[:, :].rearrange("p (h c) -> p h c", c=64)
                nc.scalar.activation(Vall[:, tt, 0, :, 0:64], vsrc, COPY)
                nc.vector.tensor_sub(
                    Vall[:, tt, 1, :, 0:64], vsrc, Vall[:, tt, 0, :, 0:64])
                # ---- K projection (transposed) ----
                kps_ = pproj.tile([P, 512], F32, tag="pp", name="kp")
                kps = kps_[:, :].rearrange("p (cg n) -> p cg n", cg=4)
                for cg in range(NCG):
                    first = True
                    for pi, (wp, xp) in enumerate(PASSES):
                        wt = w8[("d" if wp == "dw" else "") + "wk8"]
                        xt = dx8s if xp == "dx" else x8s
                        for c in range(4):
                            nc.tensor.matmul(
                                kps[:, cg, :],
                                wt[:, c, :, cg * P:(cg + 1) * P],
                                xt[:, c, :, tts],
                                start=first,
                                stop=(pi == 1 and c == 3),
                                perf_mode=DR,
                            )
                            first = False
                nc.vector.tensor_copy(KT8[:, :, 0, tts], kps)
                nc.vector.tensor_sub(
                    KT8[0:63, :, 1, tts], kps[0:63, :, :], KT8[0:63, :, 0, tts])
                nc.vector.tensor_sub(
                    KT8[64:127, :, 1, tts], kps[64:127, :, :],
                    KT8[64:127, :, 0, tts])
                if tt == 0:
                    nc.scalar.copy(K16, kps)
                # ---- Q projection (transposed) ----
                qps_ = pproj.tile([P, 512], F32, tag="pp", name="qp")
                qps = qps_[:, :].rearrange("p (cg n) -> p cg n", cg=4)
                for cg in range(NCG):
                    first = True
                    for pi, (wp, xp) in enumerate(PASSES):
                        wt = w8[("d" if wp == "dw" else "") + "wq8"]
                        xt = dx8s if xp == "dx" else x8s
                        for c in range(4):
                            nc.tensor.matmul(
                                qps[:, cg, :],
                                wt[:, c, :, cg * P:(cg + 1) * P],
                                xt[:, c, :, tts],
                                start=first,
                                stop=(pi == 1 and c == 3),
                                perf_mode=DR,
                            )
                            first = False
                nc.scalar.activation(QT8[0:64, 0:8:2, 0, tts],
                                     qps[0:64, :, :], COPY, scale=CQ)
                nc.scalar.activation(QT8[0:63, 0:8:2, 1, tts],
                                     qps[0:63, :, :], COPY, scale=CQ)
                nc.scalar.activation(QT8[64:128, 1:8:2, 0, tts],
                                     qps[64:128, :, :], COPY, scale=CQ)
                nc.scalar.activation(QT8[64:127, 1:8:2, 1, tts],
                                     qps[64:127, :, :], COPY, scale=CQ)
                if tt == 0:
                    nc.scalar.copy(Q16, qps)
                    v16src = vps[:, :].rearrange("p (h c) -> p h c", c=64)
                    nc.scalar.copy(V16[:, :, 0:64], v16src)
                # ---- attention for query block tt ----
                if tt == 0:
                    emit_attention_early(tts)
                else:
                    emit_attention(tt, tts)
                emit_outproj(tt, tts)

    return nc


# ---------------------------------------------------------------------------
# Host-side sharding / assembly
# ---------------------------------------------------------------------------

NCORES = 8
B = 4
HS = 32.0  # host weight scale


def _fold_dims(a):
    """[1024 d, N] -> [128, 4, 2, N] with d = p + 128 i + 256 c."""
    n = a.shape[1]
    return np.ascontiguousarray(
        a.reshape(4, 2, 128, n).transpose(2, 0, 1, 3))


def _f8(a):
    return np.asarray(a, np.float32).astype(NP8)


def _shard_inputs(x, Wq, Wk, Wv, Wo, bo):
    x = np.asarray(x, np.float32)
    Wq, Wk, Wv, Wo = (np.asarray(w, np.float32) for w in (Wq, Wk, Wv, Wo))

    qpos = np.arange(S)
    o_bits = A_SCH * np.sqrt(2.0 * np.log(qpos + 1.0))
    orow = _f8(o_bits / (8.0 * MCONV))[None, :]
    krow = np.full((1, S), -8.0, np.float32).astype(NP8)
    zrows = np.zeros((64, 2, S), NP8)
    jj = np.arange(256)
    kk = np.arange(128)
    rampA = (16.0 * (kk[None, :] >= jj[:, None] + 1)).astype(np.float32)
    rampB = (-224.0 * (jj[:, None] >= kk[None, :])).astype(np.float32)
    rampA8 = _f8(rampA.reshape(2, 128, 128).transpose(1, 0, 2))
    rampB8 = _f8(rampB.reshape(2, 128, 128).transpose(1, 0, 2))
    mask16 = (kk[:, None] <= kk[None, :]).astype(np.float16)  # keep k <= q

    xb8, dxb8 = [], []
    for b in range(B):
        xT = np.ascontiguousarray(x[b].T)        # [D, S]
        x8 = _f8(xT)
        dx8 = _f8(xT - x8.astype(np.float32))
        xb8.append(_fold_dims(x8))
        dxb8.append(_fold_dims(dx8))

    wmaps = []
    for g in range(2):
        cols = slice(512 * g, 512 * (g + 1))
        m = {}
        for nm, W in [("wq8", Wq), ("wk8", Wk), ("wv8", Wv)]:
            Ws = (W[:, cols] * HS).astype(np.float32)
            W8 = _f8(Ws)
            dW8 = _f8(Ws - W8.astype(np.float32))
            m[nm] = _fold_dims(W8)
            m["d" + nm] = _fold_dims(dW8)
        wo = Wo[cols, :].astype(np.float16)      # [512, 1024]
        m["wo16"] = np.ascontiguousarray(
            wo.reshape(4, 128, D).transpose(1, 0, 2))
        wmaps.append(m)

    in_maps = []
    for c in range(NCORES):
        b, g = c // 2, c % 2
        in_maps.append({
            "x8f": xb8[b], "dx8f": dxb8[b],
            **wmaps[g],
            "krow": krow, "orow": orow, "zrows": zrows,
            "rampA": rampA8, "rampB": rampB8, "mask16": mask16,
        })
    return in_maps


_NC_CACHE = {}


def _get_program():
    if "nc" not in _NC_CACHE:
        nc = build_program(num_devices=NCORES)
        nc.compile()
        _NC_CACHE["nc"] = nc
    return _NC_CACHE["nc"]


def kernel(x, Wq, Wk, Wv, Wo, bo):
    """Full-input, full-output causal MHA on 8 NeuronCores."""
    from concourse.bass_utils import run_bass_kernel_spmd

    nc = _get_program()
    in_maps = _shard_inputs(x, Wq, Wk, Wv, Wo, bo)
    res = run_bass_kernel_spmd(nc, in_maps, list(range(NCORES)))
    bo32 = np.asarray(bo, np.float32)
    out = np.zeros((B, S, D), np.float32)
    for b in range(B):
        out[b] = (res.results[2 * b]["out"].astype(np.float32)
                  + res.results[2 * b + 1]["out"].astype(np.float32)
                  + bo32[None, :])
    return out


# revision 20
# speedup vs baseline: 1.0695x; 1.0180x over previous
"""8-core Trainium2 Bass kernel for causal MHA — fp8 DoubleRow edition.

Sharding: core c = (batch c//2, head-half c%2); each core computes 8 heads over
the full 2048-token sequence and a partial output projection (contraction over
its 512 local columns); the host sums the two fp16 partials per batch + bias.

Numerics:
- Host scales all of Wq/Wk/Wv by 32 so fp8 residuals stay above e4m3's
  subnormal floor; projections run 3 DoubleRow passes (x8*W8, x8*dW8, dx8*W8)
  with fp32 PSUM accumulation.
- K stored as (K8, dK8) in the DoubleRow fold -> requant-exact scores;
  Q requantized at sigma~1.2 via a scaled copy. Slot-1 row 63/127 of K holds
  -8 and of Q holds o(q)*3.327: the matmul itself subtracts the per-query
  range offset o(q) = 11.54*sqrt(2 ln(q+1)).
- exp via the Schraudolph bit trick: one tensor_scalar (mult M, add 56) to
  uint8 on DVE/Pool, bitcast to fp8e4m3; causal masking inside the diagonal
  block via an extra fp8 "ramp" matmul accumulated into the same PSUM.
- AV in DoubleRow with the fold carrying (V8, dV8); ones-column 32.0 yields
  the softmax normalizer; context normalized to fp16.
- Query block 0 (tokens 0-127) runs a full fp16 path (its rows see no
  softmax averaging, so fp8 noise would land unattenuated).
"""

import sys
import numpy as np

if "/opt/trn_rl_repo" not in sys.path:
    sys.path.insert(0, "/opt/trn_rl_repo")

import ml_dtypes
import concourse.bass as bass
import concourse.tile as tile
from concourse import bacc, mybir

F32 = mybir.dt.float32
F16 = mybir.dt.float16
F8 = mybir.dt.float8e4
U8 = mybir.dt.uint8
EXP = mybir.ActivationFunctionType.Exp
COPY = mybir.ActivationFunctionType.Copy
DR = mybir.MatmulPerfMode.DoubleRow
ADD = mybir.AluOpType.add
MULT = mybir.AluOpType.mult
SUB = mybir.AluOpType.subtract

P = 128
S, D, HLOC, HD = 2048, 1024, 8, 64
NT = S // P                # 16 token tiles
NCG = 4                    # col groups (head pairs) per core
A_SCH = 8 * np.log2(np.e)  # 11.5416
CQ = 1.2 / 32.0            # Q8 = 1.2 q
MCONV = float(A_SCH / 8.0 / (1.2 * 32.0))   # 0.0375696
ESC = 1.0 / 8192.0         # early fp16: psum16 = 8192 s
ACTSC = float(MCONV / 11.5416023862437)   # ACT exp scale: psum*ACTSC = s - o
NP8 = ml_dtypes.float8_e4m3


def build_program(num_devices: int = 8, replicate: int = 1) -> bass.Bass:
    nc = bacc.Bacc("TRN2", target_bir_lowering=False, debug=False,
                   num_devices=num_devices)
    x8f = nc.dram_tensor("x8f", [P, 4, 2, S], F8, kind="ExternalInput")
    dx8f = nc.dram_tensor("dx8f", [P, 4, 2, S], F8, kind="ExternalInput")
    wq8 = nc.dram_tensor("wq8", [P, 4, 2, 512], F8, kind="ExternalInput")
    wk8 = nc.dram_tensor("wk8", [P, 4, 2, 512], F8, kind="ExternalInput")
    wv8 = nc.dram_tensor("wv8", [P, 4, 2, 512], F8, kind="ExternalInput")
    dwq8 = nc.dram_tensor("dwq8", [P, 4, 2, 512], F8, kind="ExternalInput")
    dwk8 = nc.dram_tensor("dwk8", [P, 4, 2, 512], F8, kind="ExternalInput")
    dwv8 = nc.dram_tensor("dwv8", [P, 4, 2, 512], F8, kind="ExternalInput")
    wo16d = nc.dram_tensor("wo16", [P, 4, D], F16, kind="ExternalInput")
    krow = nc.dram_tensor("krow", [1, S], F8, kind="ExternalInput")    # -8
    orow = nc.dram_tensor("orow", [1, S], F8, kind="ExternalInput")    # o*3.327
    zrows = nc.dram_tensor("zrows", [64, 2, S], F8, kind="ExternalInput")
    rampA = nc.dram_tensor("rampA", [P, 2, P], F8, kind="ExternalInput")
    rampB = nc.dram_tensor("rampB", [P, 2, P], F8, kind="ExternalInput")
    mask16 = nc.dram_tensor("mask16", [P, P], F16, kind="ExternalInput")
    out = nc.dram_tensor("out", [S, D], F16, kind="ExternalOutput")

    with tile.TileContext(nc) as tc:
        with (
            tc.tile_pool(name="res", bufs=1) as res,
            tc.tile_pool(name="work", bufs=4) as work,
            tc.tile_pool(name="exp8", bufs=9) as exp8,
            tc.tile_pool(name="obp", bufs=2) as obp,
            tc.tile_pool(name="pproj", bufs=2, space="PSUM") as pproj,
            tc.tile_pool(name="psc", bufs=3, space="PSUM") as psc,
            tc.tile_pool(name="pacc", bufs=2, space="PSUM") as pacc,
            tc.tile_pool(name="pout", bufs=1, space="PSUM") as pout,
        ):
          for _rep in range(replicate):
            # ---------- resident tensors ----------
            KT8 = res.tile([P, NCG, 2, S], F8, tag="kt8", name="KT8")
            QT8 = res.tile([P, HLOC, 2, S], F8, tag="qt8", name="QT8")
            Vall = res.tile([P, NT, 2, HLOC, 80], F8, tag="vall", name="Vall")
            ctx16 = res.tile([P, 4, S], F16, tag="ctx16", name="ctx16")
            x8s = res.tile([P, 4, 2, S], F8, tag="x8s", name="x8s")
            dx8s = res.tile([P, 4, 2, S], F8, tag="dx8s", name="dx8s")
            w8 = {}
            for nm in ["wq8", "wk8", "wv8", "dwq8", "dwk8", "dwv8"]:
                w8[nm] = res.tile([P, 4, 2, 512], F8, tag=nm, name=nm + "_s")
            # DMA order = first-use order: pass-1 weights + x8, then residuals,
            # then everything only needed later (wo16, attention consts).
            for nm, dt_ in [("wv8", wv8), ("wk8", wk8), ("wq8", wq8)]:
                nc.sync.dma_start(out=w8[nm], in_=dt_[:])
            nc.sync.dma_start(out=x8s, in_=x8f[:])
            for nm, dt_ in [("dwv8", dwv8), ("dwk8", dwk8), ("dwq8", dwq8)]:
                nc.sync.dma_start(out=w8[nm], in_=dt_[:])
            nc.sync.dma_start(out=dx8s, in_=dx8f[:])
            wo16 = res.tile([P, 4, D], F16, tag="wo16", name="wo16_s")
            nc.sync.dma_start(out=wo16, in_=wo16d[:])
            ramps = res.tile([P, 2, 2, P], F8, tag="ramps", name="ramps")
            nc.sync.dma_start(out=ramps[:, 0, :, :], in_=rampA[:])
            nc.sync.dma_start(out=ramps[:, 1, :, :], in_=rampB[:])
            m16 = res.tile([P, P], F16, tag="m16", name="m16")
            nc.sync.dma_start(out=m16, in_=mask16[:])
            # zero bands of QT8 (head h: other-head half zeroed), via zeros DMA
            for h in range(HLOC):
                if h % 2 == 0:
                    nc.sync.dma_start(out=QT8[64:128, h, :, :], in_=zrows[:])
                else:
                    nc.sync.dma_start(out=QT8[0:64, h, :, :], in_=zrows[:])
            # K slot1 const rows (-8) and Q slot1 o-rows
            for cg in range(NCG):
                nc.sync.dma_start(out=KT8[63:64, cg, 1, :], in_=krow[:])
                nc.sync.dma_start(out=KT8[127:128, cg, 1, :], in_=krow[:])
            for h in range(HLOC):
                r = 63 if h % 2 == 0 else 127
                nc.sync.dma_start(out=QT8[r:r + 1, h, 1, :], in_=orow[:])
            # V ones/pad columns
            nc.vector.memset(Vall[:, :, 0, :, 64:65], 32.0)
            nc.vector.memset(Vall[:, :, 1, :, 64:65], 0.0)
            nc.vector.memset(Vall[:, :, 0, :, 65:66], 0.0)
            nc.vector.memset(Vall[:, :, 1, :, 65:66], 0.0)

            negone = res.tile([P, 1], F32, tag="negone", name="negone")
            nc.vector.memset(negone, -1.0)

            # early-fp16 residents
            K16 = res.tile([P, NCG, P], F16, tag="k16", name="K16")
            Q16 = res.tile([P, NCG, P], F16, tag="q16", name="Q16")
            V16 = res.tile([P, HLOC, 65], F16, tag="v16", name="V16")
            nc.vector.memset(V16[:, :, 64:65], 32.0)

            PASSES = [("w", "x"), ("dw", "x"), ("w", "dx")]

            def proj_chains(ps, wname, tts):
                # out [128 toks, 512 cols]: lhsT = x chunk, rhs = W; 256-col halves
                for half in range(2):
                    cs = slice(half * 256, (half + 1) * 256)
                    first = True
                    for pi, (wp, xp) in enumerate(PASSES):
                        wt = w8[("d" if wp == "dw" else "") + wname]
                        xt = dx8s if xp == "dx" else x8s
                        for c in range(4):
                            nc.tensor.matmul(
                                ps[:, cs],
                                xt[:, c, :, tts],
                                wt[:, c, :, cs],
                                start=first,
                                stop=(pi == 2 and c == 3),
                                perf_mode=DR,
                            )
                            first = False

            def emit_attention(tt, tts):
                ntile = tt + 1
                ngrp = (ntile + 3) // 4
                units = [(h, g) for h in range(HLOC) for g in range(ngrp)]
                LAG = 6
                pend = []          # (h, g, sc, nj) awaiting convert
                ready = {}         # (h, g) -> (ex, nj)
                accs = {}

                def do_qk(h, g):
                    cg = h // 2
                    nj = min(4, ntile - 4 * g)
                    sc = psc.tile([P, 4, P], F32, tag="sc", name="sc")
                    for j in range(nj):
                        kt = 4 * g + j
                        nc.tensor.matmul(
                            sc[:, j, :],
                            KT8[:, cg, :, kt * P:(kt + 1) * P],
                            QT8[:, h, :, tts],
                            start=True, stop=(kt != tt), perf_mode=DR,
                        )
                        if kt == tt:
                            nc.tensor.matmul(
                                sc[:, j, :], ramps[:, 0, :, :],
                                ramps[:, 1, :, :],
                                start=False, stop=True, perf_mode=DR,
                            )
                    pend.append((h, g, sc, nj))

                def do_convert():
                    h, g, sc, nj = pend.pop(0)
                    ex = exp8.tile([P, 4, P], U8, tag="ex", name="ex")
                    if (5 * (h + 2 * tt)) % 12 < 5:
                        nc.vector.tensor_scalar(ex[:, 0:nj, :], sc[:, 0:nj, :],
                                                MCONV, 56.0, MULT, ADD)
                    else:
                        nc.scalar.activation(ex[:, 0:nj, :].bitcast(F8),
                                             sc[:, 0:nj, :], EXP,
                                             scale=ACTSC, bias=negone[:, :])
                    ready[(h, g)] = (ex, nj)

                def do_av(h, g):
                    ex, nj = ready.pop((h, g))
                    if g == 0:
                        accs[h] = pacc.tile([66, P], F32, tag="acc", name="acc")
                    acc = accs[h]
                    for j in range(nj):
                        kt = 4 * g + j
                        exd = ex[:, j, :].bitcast(F8).rearrange(
                            "p (one n) -> p one n", one=1
                        ).broadcast_to([P, 2, P])
                        nc.tensor.matmul(
                            acc, Vall[:, kt, :, h, 0:66], exd,
                            start=(kt == 0), stop=(kt == tt),
                            perf_mode=DR,
                        )
                    if 4 * g + nj == ntile:      # head complete
                        cg = h // 2
                        band0 = 0 if h % 2 == 0 else 64
                        rec = work.tile([1, P], F32, tag="rec", name="rec")
                        nc.vector.reciprocal(rec, acc[64:65, :])
                        bc = work.tile([64, P], F32, tag="bc", name="bc")
                        nc.gpsimd.partition_broadcast(bc, rec)
                        nc.vector.tensor_mul(
                            ctx16[band0:band0 + 64, cg, tts], acc[0:64, :], bc)

                for i, (h, g) in enumerate(units):
                    do_qk(h, g)
                    while pend:
                        do_convert()
                    if i >= LAG:
                        do_av(*units[i - LAG])
                while pend:
                    do_convert()
                n = len(units)
                for i in range(max(0, n - LAG), n):
                    do_av(*units[i])

            def emit_attention_early(tts):
                for h in range(HLOC):
                    cg = h // 2
                    band0 = 0 if h % 2 == 0 else 64
                    sc = psc.tile([P, 4, P], F32, tag="sc", name="sc")
                    nc.tensor.matmul(
                        sc[:, 0, :],
                        K16[band0:band0 + 64, cg, :],
                        Q16[band0:band0 + 64, cg, :],
                        start=True, stop=True,
                    )
                    exm = work.tile([P, P], F16, tag="exm", name="exm")
                    nc.scalar.activation(exm, sc[:, 0, :], EXP, scale=ESC)
                    nc.vector.tensor_mul(exm, exm, m16)
                    acc = pacc.tile([66, P], F32, tag="acc", name="acc")
                    nc.tensor.matmul(acc[0:65, :], V16[:, h, :], exm,
                                     start=True, stop=True)
                    rec = work.tile([1, P], F32, tag="rec", name="rec")
                    nc.vector.reciprocal(rec, acc[64:65, :])
                    bc = work.tile([64, P], F32, tag="bc", name="bc")
                    nc.gpsimd.partition_broadcast(bc, rec)
                    nc.vector.tensor_mul(
                        ctx16[band0:band0 + 64, cg, tts], acc[0:64, :], bc)

            def emit_outproj(tt, tts):
                for half in range(2):
                    ps = pout.tile([P, 512], F32, tag="po", name="po")
                    for ct in range(4):
                        nc.tensor.matmul(
                            ps,
                            ctx16[:, ct, tts],
                            wo16[:, ct, half * 512:(half + 1) * 512],
                            start=(ct == 0), stop=(ct == 3),
                        )
                    ob = obp.tile([P, 512], F16, tag="ob", name="ob")
                    nc.scalar.copy(ob, ps)
                    nc.sync.dma_start(
                        out=out[tt * P:(tt + 1) * P,
                                half * 512:(half + 1) * 512],
                        in_=ob,
                    )

            for tt in range(NT):
                tts = slice(tt * P, (tt + 1) * P)
                # ---- V projection ----
                vps = pproj.tile([P, 512], F32, tag="pp", name="vp")
                proj_chains(vps, "wv8", tts)
                vsrc = vps[:, :].rearrange("p (h c) -> p h c", c=64)
                nc.scalar.activation(Vall[:, tt, 0, :, 0:64], vsrc, COPY)
                nc.vector.tensor_sub(
                    Vall[:, tt, 1, :, 0:64], vsrc, Vall[:, tt, 0, :, 0:64])
                # ---- K projection (transposed) ----
                kps_ = pproj.tile([P, 512], F32, tag="pp", name="kp")
                kps = kps_[:, :].rearrange("p (cg n) -> p cg n", cg=4)
                for cg in range(NCG):
                    first = True
                    for pi, (wp, xp) in enumerate(PASSES):
                        wt = w8[("d" if wp == "dw" else "") + "wk8"]
                        xt = dx8s if xp == "dx" else x8s
                        for c in range(4):
                            nc.tensor.matmul(
                                kps[:, cg, :],
                                wt[:, c, :, cg * P:(cg + 1) * P],
                                xt[:, c, :, tts],
                                start=first,
                                stop=(pi == 1 and c == 3),
                                perf_mode=DR,
                            )
                            first = False
                nc.vector.tensor_copy(KT8[:, :, 0, tts], kps)
                nc.vector.tensor_sub(
                    KT8[0:63, :, 1, tts], kps[0:63, :, :], KT8[0:63, :, 0, tts])
                nc.vector.tensor_sub(
                    KT8[64:127, :, 1, tts], kps[64:127, :, :],
                    KT8[64:127, :, 0, tts])
                if tt == 0:
                    nc.scalar.copy(K16, kps)
                # ---- Q projection (transposed) ----
                qps_ = pproj.tile([P, 512], F32, tag="pp", name="qp")
                qps = qps_[:, :].rearrange("p (cg n) -> p cg n", cg=4)
                for cg in range(NCG):
                    first = True
                    for pi, (wp, xp) in enumerate(PASSES):
                        wt = w8[("d" if wp == "dw" else "") + "wq8"]
                        xt = dx8s if xp == "dx" else x8s
                        for c in range(4):
                            nc.tensor.matmul(
                                qps[:, cg, :],
                                wt[:, c, :, cg * P:(cg + 1) * P],
                                xt[:, c, :, tts],
                                start=first,
                                stop=(pi == 1 and c == 3),
                                perf_mode=DR,
                            )
                            first = False
                nc.scalar.activation(QT8[0:64, 0:8:2, 0, tts],
                                     qps[0:64, :, :], COPY, scale=CQ)
                nc.scalar.activation(QT8[0:63, 0:8:2, 1, tts],
                                     qps[0:63, :, :], COPY, scale=CQ)
                nc.scalar.activation(QT8[64:128, 1:8:2, 0, tts],
                                     qps[64:128, :, :], COPY, scale=CQ)
                nc.scalar.activation(QT8[64:127, 1:8:2, 1, tts],
                                     qps[64:127, :, :], COPY, scale=CQ)
                if tt == 0:
                    nc.scalar.copy(Q16, qps)
                    v16src = vps[:, :].rearrange("p (h c) -> p h c", c=64)
                    nc.scalar.copy(V16[:, :, 0:64], v16src)
                # ---- attention for query block tt ----
                if tt == 0:
                    emit_attention_early(tts)
                else:
                    emit_attention(tt, tts)
                emit_outproj(tt, tts)

    return nc


# ---------------------------------------------------------------------------
# Host-side sharding / assembly
# ---------------------------------------------------------------------------

NCORES = 8
B = 4
HS = 32.0  # host weight scale


def _fold_dims(a):
    """[1024 d, N] -> [128, 4, 2, N] with d = p + 128 i + 256 c."""
    n = a.shape[1]
    return np.ascontiguousarray(
        a.reshape(4, 2, 128, n).transpose(2, 0, 1, 3))


def _f8(a):
    return np.asarray(a, np.float32).astype(NP8)


def _shard_inputs(x, Wq, Wk, Wv, Wo, bo):
    x = np.asarray(x, np.float32)
    Wq, Wk, Wv, Wo = (np.asarray(w, np.float32) for w in (Wq, Wk, Wv, Wo))

    qpos = np.arange(S)
    o_bits = A_SCH * np.sqrt(2.0 * np.log(qpos + 1.0))
    orow = _f8(o_bits / (8.0 * MCONV))[None, :]
    krow = np.full((1, S), -8.0, np.float32).astype(NP8)
    zrows = np.zeros((64, 2, S), NP8)
    jj = np.arange(256)
    kk = np.arange(128)
    rampA = (16.0 * (kk[None, :] >= jj[:, None] + 1)).astype(np.float32)
    rampB = (-224.0 * (jj[:, None] >= kk[None, :])).astype(np.float32)
    rampA8 = _f8(rampA.reshape(2, 128, 128).transpose(1, 0, 2))
    rampB8 = _f8(rampB.reshape(2, 128, 128).transpose(1, 0, 2))
    mask16 = (kk[:, None] <= kk[None, :]).astype(np.float16)  # keep k <= q

    xb8, dxb8 = [], []
    for b in range(B):
        xT = np.ascontiguousarray(x[b].T)        # [D, S]
        x8 = _f8(xT)
        dx8 = _f8(xT - x8.astype(np.float32))
        xb8.append(_fold_dims(x8))
        dxb8.append(_fold_dims(dx8))

    wmaps = []
    for g in range(2):
        cols = slice(512 * g, 512 * (g + 1))
        m = {}
        for nm, W in [("wq8", Wq), ("wk8", Wk), ("wv8", Wv)]:
            Ws = (W[:, cols] * HS).astype(np.float32)
            W8 = _f8(Ws)
            dW8 = _f8(Ws - W8.astype(np.float32))
            m[nm] = _fold_dims(W8)
            m["d" + nm] = _fold_dims(dW8)
        wo = Wo[cols, :].astype(np.float16)      # [512, 1024]
        m["wo16"] = np.ascontiguousarray(
            wo.reshape(4, 128, D).transpose(1, 0, 2))
        wmaps.append(m)

    in_maps = []
    for c in range(NCORES):
        b, g = c // 2, c % 2
        in_maps.append({
            "x8f": xb8[b], "dx8f": dxb8[b],
            **wmaps[g],
            "krow": krow, "orow": orow, "zrows": zrows,
            "rampA": rampA8, "rampB": rampB8, "mask16": mask16,
        })
    return in_maps


_NC_CACHE = {}


def _get_program():
    if "nc" not in _NC_CACHE:
        nc = build_program(num_devices=NCORES)
        nc.compile()
        _NC_CACHE["nc"] = nc
    return _NC_CACHE["nc"]


def kernel(x, Wq, Wk, Wv, Wo, bo):
    """Full-input, full-output causal MHA on 8 NeuronCores."""
    from concourse.bass_utils import run_bass_kernel_spmd

    nc = _get_program()
    in_maps = _shard_inputs(x, Wq, Wk, Wv, Wo, bo)
    res = run_bass_kernel_spmd(nc, in_maps, list(range(NCORES)))
    bo32 = np.asarray(bo, np.float32)
    out = np.zeros((B, S, D), np.float32)
    for b in range(B):
        out[b] = (res.results[2 * b]["out"].astype(np.float32)
                  + res.results[2 * b + 1]["out"].astype(np.float32)
                  + bo32[None, :])
    return out
